# revision 7
# baseline (speedup 1.0000x reference)
"""Multi-head attention kernel for 8 Trainium2 NeuronCores.

Problem: B=4, S=2048, D=1024, H=16, Dh=64 MHA with key-side boolean mask.

Sharding: core c handles (batch b = c//2, head-half g = c%2, 8 heads each).
QKV are column-parallel, the output projection is row-parallel (Megatron
style); the host sums the two partial output projections per batch and adds
the output bias.

Host-side preprocessing (pure data marshalling):
  - All inputs are pre-tiled into DMA-native layouts (partition-major,
    contiguous per partition).
  - x is transposed per batch (the PE contracts over the partition dim).
  - Keys with mask=False contribute exactly zero after softmax, so the host
    gathers only the unmasked keys (padded to a multiple of 384 with zero
    rows whose exp-bias is -1e30 => exp == 0 exactly).
  - All matmul operands are fp16 (same PE throughput as bf16 on TRN2 but
    8x lower quantization noise; attention averages ~1e3 near-uniform keys
    so per-element noise in E/V passes straight to the output).

On-core dataflow (all matmuls fp16, PSUM accumulation fp32):
  xT --(Wk)--> KT[f,k]             bias fused in the ScalarE PSUM->SBUF copy
  xT --(Wv)--> Vau[k, h, 65]       (aug ones col -> softmax denominator)
  xT --(Wq)--> QT[f,q]
  scores[k,q] = KT_h^T x QT_h      64-deep contraction at base partition
                                   0/64 (cost is column-count bound)
  E = exp(scores*0.125 + maskbias[k])   one ScalarE pass per key tile,
                                   written to SBUF as fp16; ScalarE does
                                   nothing else during attention
  av[65,q] += Vau_kt^T x E_kt      accumulated over key tiles in PSUM
  attnT[f,q] = av[0:64] * bcast(1/av[64])  (ones-matmul broadcast + DVE)
  out[s,D] = attnT^T x Wo          (partial; host adds pair + bo)
"""

import os
import numpy as np

os.environ.setdefault("MYCRO_LOCAL_CACHE", "1")

D_MODEL = 1024
N_HEADS = 16
D_HEAD = 64
BATCH = 4
SEQ = 2048
N_CORES = 8
FH = 512          # features per core (8 heads x 64)
HPC = 8           # heads per core
NEG = -1.0e30     # additive bias for padded/masked keys; exp -> 0 exactly

F16 = np.float16

_COMPILED = {}    # k_pad -> nc
last_results = None  # BassKernelResults of the most recent run (for test.py)


def _build(k_pad):
    """Emit + compile the per-core bass kernel for a given padded key count."""
    import concourse.bacc as bacc
    import concourse.tile as tile
    from concourse import mybir

    f32 = mybir.dt.float32
    f32r = mybir.dt.float32r
    f16 = mybir.dt.float16

    KT_N = k_pad // 128                     # number of 128-key tiles
    KC = 512 if k_pad % 512 == 0 else 384   # key-side chunk
    assert k_pad % KC == 0 and KC % 128 == 0
    NKC = k_pad // KC
    HW = HPC * 65   # augmented V width (520)

    nc = bacc.Bacc("TRN2", target_bir_lowering=False, debug=False,
                   num_devices=N_CORES)

    # all pre-tiled on host into DMA-native layouts
    dxq = nc.dram_tensor("xq", [4, 128, 8, 512], f16, kind="ExternalInput")
    dxk = nc.dram_tensor("xk", [NKC, 128, 8, KC], f16, kind="ExternalInput")
    dWq = nc.dram_tensor("Wq", [128, 8, FH], f16, kind="ExternalInput")
    dWk = nc.dram_tensor("Wk", [128, 8, FH], f16, kind="ExternalInput")
    dWv = nc.dram_tensor("Wv", [128, 8, HW], f16, kind="ExternalInput")
    dWo = nc.dram_tensor("Wo", [128, 4, D_MODEL], f16, kind="ExternalInput")
    dbc = nc.dram_tensor("bcst", [128, 8 + KT_N], f32, kind="ExternalInput")
    dbv = nc.dram_tensor("bv", [HW], f16, kind="ExternalInput")
    dc16 = nc.dram_tensor("ones16", [128], f16, kind="ExternalInput")
    dcr = nc.dram_tensor("onesr", [64], f32r, kind="ExternalInput")
    dout = nc.dram_tensor("out", [SEQ, D_MODEL], f32, kind="ExternalOutput")

    EXP = mybir.ActivationFunctionType.Exp
    IDn = mybir.ActivationFunctionType.Identity

    with tile.TileContext(nc) as tc:
        with tc.tile_pool(name="persist", bufs=1) as pers:
            # ---- constants in SBUF ----
            bc = pers.tile([128, 8 + KT_N], f32, tag="bcst")
            nc.sync.dma_start(out=bc, in_=dbc.ap())
            bq = bc[:, 0:4]
            bk = bc[:, 4:8]
            mb = bc[:, 8:8 + KT_N]
            bv_row = pers.tile([1, HW], f16, tag="bvr")
            nc.sync.dma_start(out=bv_row, in_=dbv.ap()[None, :])
            ones16 = pers.tile([1, 128], f16, tag="ones16")
            nc.sync.dma_start(out=ones16, in_=dc16.ap()[None, :])
            onesr = pers.tile([1, 64], f32r, tag="onesr")
            nc.sync.dma_start(out=onesr, in_=dcr.ap()[None, :])

            # ---- persistent activations ----
            QT = pers.tile([128, 4, SEQ], f16, tag="QT")         # [f, q]
            KT = pers.tile([128, 4, k_pad], f16, tag="KT")       # [f, k]
            Vau = pers.tile([128, KT_N, HPC, 65], f16, tag="Vau")
            attnT = pers.tile([128, 4, SEQ], f16, tag="attnT")   # [f, q]
            wo = pers.tile([128, 4, D_MODEL], f16, tag="wo")

            # ================= projections =================
            wtq_cm = tc.tile_pool(name="wtq", bufs=1)
            wtq = wtq_cm.__enter__()
            wq = wtq.tile([128, 8, FH], f16, tag="wq")
            nc.sync.dma_start(out=wq, in_=dWq.ap())
            ppool_cm = tc.tile_pool(name="pp", bufs=4, space="PSUM")
            ppool = ppool_cm.__enter__()

            # ----- K side (KT, V) -----
            with tc.tile_pool(name="wtk", bufs=1) as wtk, \
                 tc.tile_pool(name="xk", bufs=2) as xkp:
                pk = ppool
                wk = wtk.tile([128, 8, FH], f16, tag="wk")
                nc.sync.dma_start(out=wk, in_=dWk.ap())
                wv = wtk.tile([128, 8, HW], f16, tag="wv")
                nc.sync.dma_start(out=wv, in_=dWv.ap())
                for kc in range(NKC):
                    xk_t = xkp.tile([128, 8, KC], f16, tag="xk")
                    nc.sync.dma_start(out=xk_t, in_=dxk.ap()[kc])
                    for ft in range(4):
                        ps = pk.tile([128, KC], f32, tag="pk")
                        for d in range(8):
                            nc.tensor.matmul(
                                ps,
                                lhsT=wk[:, d, ft * 128:(ft + 1) * 128],
                                rhs=xk_t[:, d, :],
                                start=(d == 0), stop=(d == 7))
                        ks = slice(kc * KC, (kc + 1) * KC)
                        nc.scalar.activation(KT[:, ft, ks], ps, IDn,
                                             bias=bk[:, ft:ft + 1])
                    for kb in range(KC // 128):
                        kg = kc * (KC // 128) + kb
                        ps = pk.tile([128, HW], f32, tag="pk")
                        for d in range(8):
                            lt = xk_t[:, d, kb * 128:(kb + 1) * 128]
                            nc.tensor.matmul(
                                ps[:, 0:512], lhsT=lt,
                                rhs=wv[:, d, 0:512],
                                start=(d == 0), stop=False)
                            nc.tensor.matmul(
                                ps[:, 512:520], lhsT=lt,
                                rhs=wv[:, d, 512:520],
                                start=(d == 0), stop=False)
                        nc.tensor.matmul(ps[:, 0:512], lhsT=ones16,
                                         rhs=bv_row[:, 0:512],
                                         start=False, stop=True)
                        nc.tensor.matmul(ps[:, 512:520], lhsT=ones16,
                                         rhs=bv_row[:, 512:520],
                                         start=False, stop=True)
                        nc.scalar.copy(Vau[:, kg, :, :], ps)

            # ----- Q side (QT) -----
            with tc.tile_pool(name="xq", bufs=2) as xqp:
                pq = ppool
                for qc in range(4):
                    xq_t = xqp.tile([128, 8, 512], f16, tag="xq")
                    nc.sync.dma_start(out=xq_t, in_=dxq.ap()[qc])
                    for ft in range(4):
                        ps = pq.tile([128, 512], f32, tag="pk")
                        for d in range(8):
                            nc.tensor.matmul(
                                ps,
                                lhsT=wq[:, d, ft * 128:(ft + 1) * 128],
                                rhs=xq_t[:, d, :],
                                start=(d == 0), stop=(d == 7))
                        nc.scalar.activation(QT[:, ft, qc * 512:(qc + 1) * 512],
                                             ps, IDn, bias=bq[:, ft:ft + 1])

            ppool_cm.__exit__(None, None, None)
            wtq_cm.__exit__(None, None, None)
            nc.sync.dma_start(out=wo, in_=dWo.ap())

            # ================= attention core =================
            # Per (qh, t, h): KT_N score tiles [128k, 1024q] through a
            # double-buffered PSUM pool; exp each tile straight to fp16 E
            # in SBUF; AV accumulates over key tiles in PSUM. ScalarE does
            # only exp here; everything else is PE/DVE.
            with tc.tile_pool(name="ep", bufs=2) as epl, \
                 tc.tile_pool(name="up", bufs=2) as upl, \
                 tc.tile_pool(name="rp", bufs=2) as rpl, \
                 tc.tile_pool(name="sp", bufs=2, space="PSUM") as spl, \
                 tc.tile_pool(name="av", bufs=2, space="PSUM") as avl:
                for qh in range(2):
                    q0 = qh * 1024
                    for t in range(4):
                        for h in range(2):
                            p0 = h * 64
                            hh = 2 * t + h
                            E = epl.tile([128, KT_N, 1024], f16, tag="E")
                            av = avl.tile([65, 1024], f32, tag="av")

                            def scores_exp(kt):
                                s = spl.tile([128, 1024], f32, tag="s")
                                kts = slice(kt * 128, (kt + 1) * 128)
                                for c in range(2):
                                    cs = slice(c * 512, (c + 1) * 512)
                                    qs = slice(q0 + c * 512,
                                               q0 + (c + 1) * 512)
                                    nc.tensor.matmul(
                                        s[:, cs],
                                        lhsT=KT[p0:p0 + 64, t, kts],
                                        rhs=QT[p0:p0 + 64, t, qs],
                                        start=True, stop=True)
                                nc.scalar.activation(
                                    E[:, kt, :], s, EXP,
                                    bias=mb[:, kt:kt + 1], scale=0.125)

                            def av_acc(kt):
                                for c in range(2):
                                    cs = slice(c * 512, (c + 1) * 512)
                                    nc.tensor.matmul(
                                        av[:, cs],
                                        lhsT=Vau[:, kt, hh, :],
                                        rhs=E[:, kt, cs],
                                        start=(kt == 0),
                                        stop=(kt == KT_N - 1))

                            # software pipeline: scores/exp run one tile
                            # ahead of the AV consumer
                            scores_exp(0)
                            for kt in range(KT_N):
                                if kt + 1 < KT_N:
                                    scores_exp(kt + 1)
                                av_acc(kt)

                            # normalize: attnT = av[0:64] * bcast(1/av[64])
                            r = rpl.tile([1, 1024], f32r, tag="r")
                            with nc.allow_low_precision(
                                    reason="fp32r matmul operand"):
                                nc.vector.reciprocal(r, av[64:65, :])
                            bcs = spl.tile([64, 1024], f32, tag="s")
                            for c in range(2):
                                cs = slice(c * 512, (c + 1) * 512)
                                nc.tensor.matmul(bcs[:, cs], lhsT=onesr,
                                                 rhs=r[:, cs],
                                                 start=True, stop=True)
                            u = upl.tile([64, 1024], f16, tag="u")
                            with nc.allow_low_precision(
                                    reason="fp16 attn staging"):
                                nc.vector.tensor_copy(u, av[0:64, :])
                                nc.vector.tensor_mul(
                                    attnT[p0:p0 + 64, t, q0:q0 + 1024],
                                    u, bcs)

            # ================= output projection (partial) =================
            with tc.tile_pool(name="op", bufs=2, space="PSUM") as opp, \
                 tc.tile_pool(name="ot", bufs=3) as otp:
                for st in range(16):
                    ps = opp.tile([128, D_MODEL], f32, tag="op")
                    for ft in range(4):
                        for dh in range(2):
                            nc.tensor.matmul(
                                ps[:, dh * 512:(dh + 1) * 512],
                                lhsT=attnT[:, ft, st * 128:(st + 1) * 128],
                                rhs=wo[:, ft, dh * 512:(dh + 1) * 512],
                                start=(ft == 0), stop=(ft == 3))
                    ot = otp.tile([128, D_MODEL], f32, tag="ot")
                    nc.vector.tensor_copy(ot, ps)
                    nc.sync.dma_start(out=dout.ap()[st * 128:(st + 1) * 128, :],
                                      in_=ot)

    nc.compile()
    return nc


def _get_compiled(k_pad):
    if k_pad not in _COMPILED:
        _COMPILED[k_pad] = _build(k_pad)
    return _COMPILED[k_pad]


def _tile_pf(a, p=128):
    """[P*t, f...] -> contiguous [p, t, f...] partition-major tiling."""
    t = a.shape[0] // p
    return np.ascontiguousarray(
        a.reshape(t, p, *a.shape[1:]).swapaxes(0, 1))


def _prep_core_inputs(x, attention_mask, Wq, bq, Wk, bk, Wv, bv, Wo):
    """Host-side shard prep. Returns (in_maps, k_pad)."""
    x = np.asarray(x, np.float32)
    mask = np.asarray(attention_mask, bool)
    idxs = [np.nonzero(mask[b])[0] for b in range(BATCH)]
    ke_max = max(1, max(len(i) for i in idxs))
    k_pad = 384 * ((ke_max + 383) // 384)
    if k_pad > SEQ:
        k_pad = SEQ
    KC = 512 if k_pad % 512 == 0 else 384
    NKC = k_pad // KC
    KT_N = k_pad // 128

    ones16 = np.ones(128, F16)
    onesr = np.ones(64, np.float32)

    in_maps = []
    for b in range(BATCH):
        xT = x[b].T                                  # [D, S] view
        # xq: [qc, p, dt, 512]
        xq = np.ascontiguousarray(
            xT.reshape(8, 128, 4, 512).transpose(2, 1, 0, 3)).astype(F16)
        idx = idxs[b]
        ke = len(idx)
        if ke > k_pad:
            idx = idx[:k_pad]
            ke = k_pad
        xkT = np.zeros((D_MODEL, k_pad), np.float32)
        xkT[:, :ke] = x[b][idx].T
        # xk: [kc, p, dt, KC]
        xk = np.ascontiguousarray(
            xkT.reshape(8, 128, NKC, KC).transpose(2, 1, 0, 3)).astype(F16)
        maskb = np.zeros(k_pad, np.float32)
        maskb[ke:] = NEG
        mb_t = _tile_pf(maskb)                       # [128, KT_N]
        for g in range(2):
            fs = slice(g * FH, (g + 1) * FH)
            # Wv/bv padded with a ones column per head: the V-projection
            # matmul then produces [V_h | ones] directly (col = 0*x + 1.0).
            Wv_aug = np.zeros((D_MODEL, HPC * 65), np.float32)
            bv_aug = np.zeros(HPC * 65, np.float32)
            for h in range(HPC):
                Wv_aug[:, h * 65:h * 65 + 64] = Wv[:, g * FH + h * 64:
                                                   g * FH + (h + 1) * 64]
                bv_aug[h * 65:h * 65 + 64] = bv[g * FH + h * 64:
                                                g * FH + (h + 1) * 64]
                bv_aug[h * 65 + 64] = 1.0
            in_maps.append({
                "xq": xq,
                "xk": xk,
                "Wq": _tile_pf(np.asarray(Wq[:, fs], np.float32)).astype(F16),
                "Wk": _tile_pf(np.asarray(Wk[:, fs], np.float32)).astype(F16),
                "Wv": _tile_pf(Wv_aug).astype(F16),
                "Wo": _tile_pf(np.asarray(Wo[fs, :], np.float32)).astype(F16),
                "bcst": np.concatenate(
                    [_tile_pf(np.asarray(bq[fs], np.float32)),
                     _tile_pf(np.asarray(bk[fs], np.float32)),
                     mb_t], axis=1).astype(np.float32),
                "bv": bv_aug.astype(F16),
                "ones16": ones16,
                "onesr": onesr,
            })
    return in_maps, k_pad


def kernel(x, attention_mask, Wq, bq, Wk, bk, Wv, bv, Wo, bo):
    global last_results
    from concourse.bass_utils import run_bass_kernel_spmd

    in_maps, k_pad = _prep_core_inputs(x, attention_mask, Wq, bq, Wk, bk,
                                       Wv, bv, Wo)
    nc = _get_compiled(k_pad)
    res = run_bass_kernel_spmd(nc, in_maps, core_ids=list(range(N_CORES)))
    last_results = res

    bo = np.asarray(bo, np.float32)
    out = np.empty((BATCH, SEQ, D_MODEL), np.float32)
    for b in range(BATCH):
        out[b] = res.results[2 * b]["out"] + res.results[2 * b + 1]["out"] + bo
    return out


# revision 12
# speedup vs baseline: 1.1390x; 1.1390x over previous
"""Multi-head attention kernel for 8 Trainium2 NeuronCores.

Problem: B=4, S=2048, D=1024, H=16, Dh=64 MHA with key-side boolean mask.

Sharding: core c handles (batch b = c//2, head-half g = c%2, 8 heads each).
QKV are column-parallel, the output projection is row-parallel (Megatron
style); the host sums the two partial output projections per batch and adds
the output bias.

Host-side preprocessing (pure data marshalling):
  - All inputs are pre-tiled into DMA-native layouts (partition-major,
    contiguous per partition).
  - x is transposed per batch (the PE contracts over the partition dim).
  - Keys with mask=False contribute exactly zero after softmax, so the host
    gathers only the unmasked keys (padded to a multiple of 384 with zero
    rows whose exp-bias is -1e30 => exp == 0 exactly).
  - All matmul operands are fp16 (same PE throughput as bf16 on TRN2 but
    8x lower quantization noise; attention averages ~1e3 near-uniform keys
    so per-element noise in E/V passes straight to the output).

On-core dataflow (all matmuls fp16, PSUM accumulation fp32):
  xT --(Wk)--> KT[f,k]             bias fused in the ScalarE PSUM->SBUF copy
  xT --(Wv)--> Vau[k, h, 65]       (aug ones col -> softmax denominator)
  xT --(Wq)--> QT[f,q]
  scores[k,q] = KT_h^T x QT_h      64-deep contraction at base partition
                                   0/64 (cost is column-count bound)
  E = exp(scores*0.125 + maskbias[k])   one ScalarE pass per key tile,
                                   written to SBUF as fp16; ScalarE does
                                   nothing else during attention
  av[65,q] += Vau_kt^T x E_kt      accumulated over key tiles in PSUM
  attnT[f,q] = av[0:64] * bcast(1/av[64])  (ones-matmul broadcast + DVE)
  out[s,D] = attnT^T x Wo          (partial; host adds pair + bo)
"""

import os
import numpy as np

os.environ.setdefault("MYCRO_LOCAL_CACHE", "1")

D_MODEL = 1024
N_HEADS = 16
D_HEAD = 64
BATCH = 4
SEQ = 2048
N_CORES = 8
FH = 512          # features per core (8 heads x 64)
HPC = 8           # heads per core
NEG = -1.0e30     # additive bias for padded/masked keys; exp -> 0 exactly

F16 = np.float16

_COMPILED = {}    # k_pad -> nc
last_results = None  # BassKernelResults of the most recent run (for test.py)


def _build(k_pad):
    """Emit + compile the per-core bass kernel for a given padded key count."""
    import concourse.bacc as bacc
    import concourse.tile as tile
    from concourse import mybir

    f32 = mybir.dt.float32
    f32r = mybir.dt.float32r
    f16 = mybir.dt.float16

    KT_N = k_pad // 128                     # number of 128-key tiles
    KC = 512 if k_pad % 512 == 0 else 384   # key-side chunk
    assert k_pad % KC == 0 and KC % 128 == 0
    NKC = k_pad // KC
    HW = HPC * 65   # augmented V width (520)

    nc = bacc.Bacc("TRN2", target_bir_lowering=False, debug=False,
                   num_devices=N_CORES)

    # all pre-tiled on host into DMA-native layouts
    dxq = nc.dram_tensor("xq", [4, 128, 8, 512], f16, kind="ExternalInput")
    dxk = nc.dram_tensor("xk", [NKC, 128, 8, KC], f16, kind="ExternalInput")
    dWq = nc.dram_tensor("Wq", [128, 8, FH], f16, kind="ExternalInput")
    dWk = nc.dram_tensor("Wk", [128, 8, FH], f16, kind="ExternalInput")
    dWv = nc.dram_tensor("Wv", [128, 8, HW], f16, kind="ExternalInput")
    dWo = nc.dram_tensor("Wo", [128, 4, D_MODEL], f16, kind="ExternalInput")
    dbc = nc.dram_tensor("bcst", [128, 8 + KT_N], f32, kind="ExternalInput")
    dbv = nc.dram_tensor("bv", [HW], f16, kind="ExternalInput")
    dc16 = nc.dram_tensor("ones16", [128], f16, kind="ExternalInput")
    dout = nc.dram_tensor("out", [SEQ, D_MODEL], f32, kind="ExternalOutput")

    EXP = mybir.ActivationFunctionType.Exp
    IDn = mybir.ActivationFunctionType.Identity

    with tile.TileContext(nc) as tc:
        with tc.tile_pool(name="persist", bufs=1) as pers:
            # ---- constants in SBUF ----
            bc = pers.tile([128, 8 + KT_N], f32, tag="bcst")
            nc.sync.dma_start(out=bc, in_=dbc.ap())
            bq = bc[:, 0:4]
            bk = bc[:, 4:8]
            mb = bc[:, 8:8 + KT_N]
            bv_row = pers.tile([1, HW], f16, tag="bvr")
            nc.sync.dma_start(out=bv_row, in_=dbv.ap()[None, :])
            ones16 = pers.tile([1, 128], f16, tag="ones16")
            nc.sync.dma_start(out=ones16, in_=dc16.ap()[None, :])

            # ---- persistent activations ----
            QT = pers.tile([128, 4, SEQ], f16, tag="QT")         # [f, q]
            KT = pers.tile([128, 4, k_pad], f16, tag="KT")       # [f, k]
            Vau = pers.tile([128, KT_N, HPC, 65], f16, tag="Vau")
            attnT = pers.tile([128, 4, SEQ], f16, tag="attnT")   # [f, q]
            wo = pers.tile([128, 4, D_MODEL], f16, tag="wo")

            # ================= projections =================
            wtq_cm = tc.tile_pool(name="wtq", bufs=1)
            wtq = wtq_cm.__enter__()
            wq = wtq.tile([128, 8, FH], f16, tag="wq")
            nc.sync.dma_start(out=wq, in_=dWq.ap())
            ppool_cm = tc.tile_pool(name="pp", bufs=4, space="PSUM")
            ppool = ppool_cm.__enter__()

            # ----- K side (KT, V) -----
            with tc.tile_pool(name="wtk", bufs=1) as wtk, \
                 tc.tile_pool(name="xk", bufs=2) as xkp:
                pk = ppool
                wk = wtk.tile([128, 8, FH], f16, tag="wk")
                nc.sync.dma_start(out=wk, in_=dWk.ap())
                wv = wtk.tile([128, 8, HW], f16, tag="wv")
                nc.sync.dma_start(out=wv, in_=dWv.ap())
                for kc in range(NKC):
                    xk_t = xkp.tile([128, 8, KC], f16, tag="xk")
                    nc.sync.dma_start(out=xk_t, in_=dxk.ap()[kc])
                    for ft in range(4):
                        ps = pk.tile([128, KC], f32, tag="pk")
                        for d in range(8):
                            nc.tensor.matmul(
                                ps,
                                lhsT=wk[:, d, ft * 128:(ft + 1) * 128],
                                rhs=xk_t[:, d, :],
                                start=(d == 0), stop=(d == 7))
                        ks = slice(kc * KC, (kc + 1) * KC)
                        nc.scalar.activation(KT[:, ft, ks], ps, IDn,
                                             bias=bk[:, ft:ft + 1])
                    for kb in range(KC // 128):
                        kg = kc * (KC // 128) + kb
                        ps = pk.tile([128, HW], f32, tag="pk")
                        for d in range(8):
                            lt = xk_t[:, d, kb * 128:(kb + 1) * 128]
                            nc.tensor.matmul(
                                ps[:, 0:512], lhsT=lt,
                                rhs=wv[:, d, 0:512],
                                start=(d == 0), stop=False)
                            nc.tensor.matmul(
                                ps[:, 512:520], lhsT=lt,
                                rhs=wv[:, d, 512:520],
                                start=(d == 0), stop=False)
                        nc.tensor.matmul(ps[:, 0:512], lhsT=ones16,
                                         rhs=bv_row[:, 0:512],
                                         start=False, stop=True)
                        nc.tensor.matmul(ps[:, 512:520], lhsT=ones16,
                                         rhs=bv_row[:, 512:520],
                                         start=False, stop=True)
                        nc.scalar.copy(Vau[:, kg, :, :], ps)

            # ----- Q side (QT) -----
            with tc.tile_pool(name="xq", bufs=2) as xqp:
                pq = ppool
                for qc in range(4):
                    xq_t = xqp.tile([128, 8, 512], f16, tag="xq")
                    nc.sync.dma_start(out=xq_t, in_=dxq.ap()[qc])
                    for ft in range(4):
                        ps = pq.tile([128, 512], f32, tag="pk")
                        for d in range(8):
                            nc.tensor.matmul(
                                ps,
                                lhsT=wq[:, d, ft * 128:(ft + 1) * 128],
                                rhs=xq_t[:, d, :],
                                start=(d == 0), stop=(d == 7))
                        nc.scalar.activation(QT[:, ft, qc * 512:(qc + 1) * 512],
                                             ps, IDn, bias=bq[:, ft:ft + 1])

            ppool_cm.__exit__(None, None, None)
            wtq_cm.__exit__(None, None, None)
            nc.sync.dma_start(out=wo, in_=dWo.ap())

            # ================= attention core =================
            # Per (qh, t, h): KT_N score tiles [128k, 1024q] through a
            # double-buffered PSUM pool; exp each tile straight to fp16 E
            # in SBUF; AV accumulates over key tiles in PSUM. ScalarE does
            # only exp here. O-projection matmuls for the finished query
            # half are woven one-per-key-tile-slot into the other half's
            # attention loop, keeping the PE busy (full p-state) while it
            # would otherwise wait on ScalarE.
            with tc.tile_pool(name="ep", bufs=2) as epl, \
                 tc.tile_pool(name="up", bufs=2) as upl, \
                 tc.tile_pool(name="rp", bufs=2) as rpl, \
                 tc.tile_pool(name="sp", bufs=2, space="PSUM") as spl, \
                 tc.tile_pool(name="av", bufs=1, space="PSUM") as avl, \
                 tc.tile_pool(name="op", bufs=2, space="PSUM") as opl, \
                 tc.tile_pool(name="ot", bufs=3) as otl:

                def oproj_tile(st):
                    """Yield (emit-)closures: 8 matmul slots + finalizers."""
                    sts = slice(st * 128, (st + 1) * 128)
                    ps = [None, None]

                    def mk_mm(dh, ft):
                        def mm():
                            if ft == 0:
                                ps[dh] = opl.tile([128, 512], f32, tag="op",
                                                  name=f"ops{st}_{dh}")
                            nc.tensor.matmul(
                                ps[dh],
                                lhsT=attnT[:, ft, sts],
                                rhs=wo[:, ft, dh * 512:(dh + 1) * 512],
                                start=(ft == 0), stop=(ft == 3))
                            if ft == 3:
                                ot = otl.tile([128, 512], f32, tag="ot")
                                nc.vector.tensor_copy(ot, ps[dh])
                                nc.sync.dma_start(
                                    out=dout.ap()[sts,
                                                  dh * 512:(dh + 1) * 512],
                                    in_=ot)
                        return mm

                    return [mk_mm(dh, ft) for dh in range(2)
                            for ft in range(4)]

                def attn_head(qh, t, h, weave):
                    q0 = qh * 1024
                    p0 = h * 64
                    hh = 2 * t + h
                    E = epl.tile([128, KT_N, 1024], f16, tag="E")
                    av = avl.tile([65, 1024], f32, tag="av")

                    def scores_exp(kt):
                        s = spl.tile([128, 1024], f32, tag="s")
                        kts = slice(kt * 128, (kt + 1) * 128)
                        for c in range(2):
                            cs = slice(c * 512, (c + 1) * 512)
                            qs = slice(q0 + c * 512, q0 + (c + 1) * 512)
                            nc.tensor.matmul(
                                s[:, cs],
                                lhsT=KT[p0:p0 + 64, t, kts],
                                rhs=QT[p0:p0 + 64, t, qs],
                                start=True, stop=True)
                        nc.scalar.activation(
                            E[:, kt, :], s, EXP,
                            bias=mb[:, kt:kt + 1], scale=0.125)

                    # software pipeline: scores/exp run one tile ahead of
                    # the AV consumer; weave ops fill the exp-wait gap
                    scores_exp(0)
                    wv_i = 0
                    wv_n = len(weave)
                    for kt in range(KT_N):
                        if kt + 1 < KT_N:
                            scores_exp(kt + 1)
                        while wv_i < wv_n and wv_i * KT_N < (kt + 1) * wv_n:
                            weave[wv_i]()
                            wv_i += 1
                        for c in range(2):
                            cs = slice(c * 512, (c + 1) * 512)
                            nc.tensor.matmul(
                                av[:, cs],
                                lhsT=Vau[:, kt, hh, :],
                                rhs=E[:, kt, cs],
                                start=(kt == 0),
                                stop=(kt == KT_N - 1))

                    # normalize: attnT = av[0:64] * bcast(1/av[64])
                    dn = rpl.tile([1, 1024], f32, tag="dn")
                    nc.vector.tensor_copy(dn, av[64:65, :])
                    rf = rpl.tile([1, 1024], f32, tag="rf")
                    nc.vector.reciprocal_approx_fast(out=rf, in_=dn)
                    r16 = rpl.tile([1, 1024], f16, tag="r16")
                    with nc.allow_low_precision(reason="fp16 recip"):
                        nc.vector.tensor_copy(r16, rf)
                    bcs = spl.tile([64, 1024], f32, tag="s")
                    for c in range(2):
                        cs = slice(c * 512, (c + 1) * 512)
                        nc.tensor.matmul(bcs[:, cs], lhsT=ones16[:, 0:64],
                                         rhs=r16[:, cs],
                                         start=True, stop=True)
                    u = upl.tile([64, 1024], f16, tag="u")
                    with nc.allow_low_precision(
                            reason="fp16 attn staging"):
                        nc.vector.tensor_copy(u, av[0:64, :])
                        nc.vector.tensor_mul(
                            attnT[p0:p0 + 64, t, q0:q0 + 1024],
                            u, bcs)

                for t in range(4):
                    for h in range(2):
                        attn_head(0, t, h, [])
                pending = []
                for i, (t, h) in enumerate(
                        [(t, h) for t in range(4) for h in range(2)]):
                    pending += oproj_tile(i)      # q-half 0 output tiles
                    attn_head(1, t, h, pending)
                    pending = []
                # tail: q-half 1 output tiles
                for st in range(8, 16):
                    for mm in oproj_tile(st):
                        mm()

    nc.compile()
    return nc


def _get_compiled(k_pad):
    if k_pad not in _COMPILED:
        _COMPILED[k_pad] = _build(k_pad)
    return _COMPILED[k_pad]


def _tile_pf(a, p=128):
    """[P*t, f...] -> contiguous [p, t, f...] partition-major tiling."""
    t = a.shape[0] // p
    return np.ascontiguousarray(
        a.reshape(t, p, *a.shape[1:]).swapaxes(0, 1))


def _prep_core_inputs(x, attention_mask, Wq, bq, Wk, bk, Wv, bv, Wo):
    """Host-side shard prep. Returns (in_maps, k_pad)."""
    x = np.asarray(x, np.float32)
    mask = np.asarray(attention_mask, bool)
    idxs = [np.nonzero(mask[b])[0] for b in range(BATCH)]
    ke_max = max(1, max(len(i) for i in idxs))
    k_pad = 384 * ((ke_max + 383) // 384)
    if k_pad > SEQ:
        k_pad = SEQ
    KC = 512 if k_pad % 512 == 0 else 384
    NKC = k_pad // KC
    KT_N = k_pad // 128

    ones16 = np.ones(128, F16)

    in_maps = []
    for b in range(BATCH):
        xT = x[b].T                                  # [D, S] view
        # xq: [qc, p, dt, 512]
        xq = np.ascontiguousarray(
            xT.reshape(8, 128, 4, 512).transpose(2, 1, 0, 3)).astype(F16)
        idx = idxs[b]
        ke = len(idx)
        if ke > k_pad:
            idx = idx[:k_pad]
            ke = k_pad
        xkT = np.zeros((D_MODEL, k_pad), np.float32)
        xkT[:, :ke] = x[b][idx].T
        # xk: [kc, p, dt, KC]
        xk = np.ascontiguousarray(
            xkT.reshape(8, 128, NKC, KC).transpose(2, 1, 0, 3)).astype(F16)
        maskb = np.zeros(k_pad, np.float32)
        maskb[ke:] = NEG
        mb_t = _tile_pf(maskb)                       # [128, KT_N]
        for g in range(2):
            fs = slice(g * FH, (g + 1) * FH)
            # Wv/bv padded with a ones column per head: the V-projection
            # matmul then produces [V_h | ones] directly (col = 0*x + 1.0).
            Wv_aug = np.zeros((D_MODEL, HPC * 65), np.float32)
            bv_aug = np.zeros(HPC * 65, np.float32)
            for h in range(HPC):
                Wv_aug[:, h * 65:h * 65 + 64] = Wv[:, g * FH + h * 64:
                                                   g * FH + (h + 1) * 64]
                bv_aug[h * 65:h * 65 + 64] = bv[g * FH + h * 64:
                                                g * FH + (h + 1) * 64]
                bv_aug[h * 65 + 64] = 1.0
            in_maps.append({
                "xq": xq,
                "xk": xk,
                "Wq": _tile_pf(np.asarray(Wq[:, fs], np.float32)).astype(F16),
                "Wk": _tile_pf(np.asarray(Wk[:, fs], np.float32)).astype(F16),
                "Wv": _tile_pf(Wv_aug).astype(F16),
                "Wo": _tile_pf(np.asarray(Wo[fs, :], np.float32)).astype(F16),
                "bcst": np.concatenate(
                    [_tile_pf(np.asarray(bq[fs], np.float32)),
                     _tile_pf(np.asarray(bk[fs], np.float32)),
                     mb_t], axis=1).astype(np.float32),
                "bv": bv_aug.astype(F16),
                "ones16": ones16,
            })
    return in_maps, k_pad


def kernel(x, attention_mask, Wq, bq, Wk, bk, Wv, bv, Wo, bo):
    global last_results
    from concourse.bass_utils import run_bass_kernel_spmd

    in_maps, k_pad = _prep_core_inputs(x, attention_mask, Wq, bq, Wk, bk,
                                       Wv, bv, Wo)
    nc = _get_compiled(k_pad)
    res = run_bass_kernel_spmd(nc, in_maps, core_ids=list(range(N_CORES)))
    last_results = res

    bo = np.asarray(bo, np.float32)
    out = np.empty((BATCH, SEQ, D_MODEL), np.float32)
    for b in range(BATCH):
        out[b] = res.results[2 * b]["out"] + res.results[2 * b + 1]["out"] + bo
    return out


# revision 14
# speedup vs baseline: 1.1882x; 1.0432x over previous
"""Multi-head attention kernel for 8 Trainium2 NeuronCores.

Problem: B=4, S=2048, D=1024, H=16, Dh=64 MHA with key-side boolean mask.

Sharding: core c handles (batch b = c//2, head-half g = c%2, 8 heads each).
QKV are column-parallel, the output projection is row-parallel (Megatron
style); the host sums the two partial output projections per batch and adds
the output bias.

Host-side preprocessing (pure data marshalling):
  - All inputs are pre-tiled into DMA-native layouts (partition-major,
    contiguous per partition).
  - x is transposed per batch (the PE contracts over the partition dim).
  - Keys with mask=False contribute exactly zero after softmax, so the host
    gathers only the unmasked keys (padded to a multiple of 384 with zero
    rows whose exp-bias is -1e30 => exp == 0 exactly).
  - All matmul operands are fp16 (same PE throughput as bf16 on TRN2 but
    8x lower quantization noise; attention averages ~1e3 near-uniform keys
    so per-element noise in E/V passes straight to the output).

On-core dataflow (all matmuls fp16, PSUM accumulation fp32):
  xT --(Wk)--> KT[f,k]             bias fused in the ScalarE PSUM->SBUF copy
  xT --(Wv)--> Vau[k, h, 65]       (aug ones col -> softmax denominator)
  xT --(Wq)--> QT[f,q]
  scores[k,q] = KT_h^T x QT_h      64-deep contraction at base partition
                                   0/64 (cost is column-count bound)
  E = exp(scores*0.125 + maskbias[k])   one ScalarE pass per key tile,
                                   written to SBUF as fp16; ScalarE does
                                   nothing else during attention
  av[65,q] += Vau_kt^T x E_kt      accumulated over key tiles in PSUM
  attnT[f,q] = av[0:64] * bcast(1/av[64])  (ones-matmul broadcast + DVE)
  out[s,D] = attnT^T x Wo          (partial; host adds pair + bo)
"""

import os
import numpy as np

os.environ.setdefault("MYCRO_LOCAL_CACHE", "1")

D_MODEL = 1024
N_HEADS = 16
D_HEAD = 64
BATCH = 4
SEQ = 2048
N_CORES = 8
FH = 512          # features per core (8 heads x 64)
HPC = 8           # heads per core
NEG = -1.0e30     # additive bias for padded/masked keys; exp -> 0 exactly

F16 = np.float16

_COMPILED = {}    # k_pad -> nc
last_results = None  # BassKernelResults of the most recent run (for test.py)


def _build(k_pad):
    """Emit + compile the per-core bass kernel for a given padded key count."""
    import concourse.bacc as bacc
    import concourse.tile as tile
    from concourse import mybir

    f32 = mybir.dt.float32
    f32r = mybir.dt.float32r
    f16 = mybir.dt.float16

    KT_N = k_pad // 128                     # number of 128-key tiles
    KC = 512 if k_pad % 512 == 0 else 384   # key-side chunk
    assert k_pad % KC == 0 and KC % 128 == 0
    NKC = k_pad // KC
    HW = HPC * 65   # augmented V width (520)

    nc = bacc.Bacc("TRN2", target_bir_lowering=False, debug=False,
                   num_devices=N_CORES)

    # all pre-tiled on host into DMA-native layouts
    dxq = nc.dram_tensor("xq", [4, 128, 8, 512], f16, kind="ExternalInput")
    dxk = nc.dram_tensor("xk", [NKC, 128, 8, KC], f16, kind="ExternalInput")
    dWq = nc.dram_tensor("Wq", [128, 8, FH], f16, kind="ExternalInput")
    dWk = nc.dram_tensor("Wk", [128, 8, FH], f16, kind="ExternalInput")
    dWv = nc.dram_tensor("Wv", [128, 8, HW], f16, kind="ExternalInput")
    dWo = nc.dram_tensor("Wo", [128, 4, D_MODEL], f16, kind="ExternalInput")
    dbc = nc.dram_tensor("bcst", [128, 8 + KT_N], f32, kind="ExternalInput")
    dbv = nc.dram_tensor("bv", [HW], f16, kind="ExternalInput")
    dc16 = nc.dram_tensor("ones16", [128], f16, kind="ExternalInput")
    dout = nc.dram_tensor("out", [SEQ, D_MODEL], f32, kind="ExternalOutput")

    EXP = mybir.ActivationFunctionType.Exp
    IDn = mybir.ActivationFunctionType.Identity

    with tile.TileContext(nc) as tc:
        with tc.tile_pool(name="persist", bufs=1) as pers:
            # ---- constants in SBUF ----
            bc = pers.tile([128, 8 + KT_N], f32, tag="bcst")
            nc.sync.dma_start(out=bc, in_=dbc.ap())
            bq = bc[:, 0:4]
            bk = bc[:, 4:8]
            mb = bc[:, 8:8 + KT_N]
            bv_row = pers.tile([1, HW], f16, tag="bvr")
            nc.sync.dma_start(out=bv_row, in_=dbv.ap()[None, :])
            ones16 = pers.tile([1, 128], f16, tag="ones16")
            nc.sync.dma_start(out=ones16, in_=dc16.ap()[None, :])

            # ---- persistent activations ----
            QT = pers.tile([128, 4, SEQ], f16, tag="QT")         # [f, q]
            KT = pers.tile([128, 4, k_pad], f16, tag="KT")       # [f, k]
            Vau = pers.tile([128, KT_N, HPC, 65], f16, tag="Vau")
            attnT = pers.tile([128, 4, SEQ], f16, tag="attnT")   # [f, q]
            wo = pers.tile([128, 4, D_MODEL], f16, tag="wo")

            # ================= projections =================
            wq = pers.tile([128, 8, FH], f16, tag="wq")
            nc.sync.dma_start(out=wq, in_=dWq.ap())
            xq2a = pers.tile([128, 8, 512], f16, tag="xq2a")
            nc.sync.dma_start(out=xq2a, in_=dxq.ap()[2])
            xq2b = pers.tile([128, 8, 512], f16, tag="xq2b")
            nc.sync.dma_start(out=xq2b, in_=dxq.ap()[3])
            xq2 = {2: xq2a, 3: xq2b}
            ppool_cm = tc.tile_pool(name="pp", bufs=4, space="PSUM")
            ppool = ppool_cm.__enter__()

            # ----- K side (KT, V) -----
            with tc.tile_pool(name="wtk", bufs=1) as wtk, \
                 tc.tile_pool(name="xk", bufs=2) as xkp:
                pk = ppool
                wk = wtk.tile([128, 8, FH], f16, tag="wk")
                nc.sync.dma_start(out=wk, in_=dWk.ap())
                wv = wtk.tile([128, 8, HW], f16, tag="wv")
                nc.sync.dma_start(out=wv, in_=dWv.ap())
                for kc in range(NKC):
                    xk_t = xkp.tile([128, 8, KC], f16, tag="xk")
                    nc.sync.dma_start(out=xk_t, in_=dxk.ap()[kc])
                    for ft in range(4):
                        ps = pk.tile([128, KC], f32, tag="pk")
                        for d in range(8):
                            nc.tensor.matmul(
                                ps,
                                lhsT=wk[:, d, ft * 128:(ft + 1) * 128],
                                rhs=xk_t[:, d, :],
                                start=(d == 0), stop=(d == 7))
                        ks = slice(kc * KC, (kc + 1) * KC)
                        nc.scalar.activation(KT[:, ft, ks], ps, IDn,
                                             bias=bk[:, ft:ft + 1])
                    for kb in range(KC // 128):
                        kg = kc * (KC // 128) + kb
                        ps = pk.tile([128, HW], f32, tag="pk")
                        for d in range(8):
                            lt = xk_t[:, d, kb * 128:(kb + 1) * 128]
                            nc.tensor.matmul(
                                ps[:, 0:512], lhsT=lt,
                                rhs=wv[:, d, 0:512],
                                start=(d == 0), stop=False)
                            nc.tensor.matmul(
                                ps[:, 512:520], lhsT=lt,
                                rhs=wv[:, d, 512:520],
                                start=(d == 0), stop=False)
                        nc.tensor.matmul(ps[:, 0:512], lhsT=ones16,
                                         rhs=bv_row[:, 0:512],
                                         start=False, stop=True)
                        nc.tensor.matmul(ps[:, 512:520], lhsT=ones16,
                                         rhs=bv_row[:, 512:520],
                                         start=False, stop=True)
                        nc.scalar.copy(Vau[:, kg, :, :], ps)

            # ----- Q side (QT): qc 0,1 here; qc 2,3 woven into the
            # qh=0 attention loop (their queries are only read in qh=1)
            with tc.tile_pool(name="xq", bufs=2) as xqp:
                pq = ppool
                for qc in range(2):
                    xq_t = xqp.tile([128, 8, 512], f16, tag="xq")
                    nc.sync.dma_start(out=xq_t, in_=dxq.ap()[qc])
                    for ft in range(4):
                        ps = pq.tile([128, 512], f32, tag="pk")
                        for d in range(8):
                            nc.tensor.matmul(
                                ps,
                                lhsT=wq[:, d, ft * 128:(ft + 1) * 128],
                                rhs=xq_t[:, d, :],
                                start=(d == 0), stop=(d == 7))
                        nc.scalar.activation(QT[:, ft, qc * 512:(qc + 1) * 512],
                                             ps, IDn, bias=bq[:, ft:ft + 1])

            ppool_cm.__exit__(None, None, None)
            nc.sync.dma_start(out=wo, in_=dWo.ap())

            # ================= attention core =================
            # Per (qh, t, h): KT_N score tiles [128k, 1024q] through a
            # double-buffered PSUM pool; exp each tile straight to fp16 E
            # in SBUF; AV accumulates over key tiles in PSUM. ScalarE does
            # only exp here. O-projection matmuls for the finished query
            # half are woven one-per-key-tile-slot into the other half's
            # attention loop, keeping the PE busy (full p-state) while it
            # would otherwise wait on ScalarE.
            with tc.tile_pool(name="ep", bufs=2) as epl, \
                 tc.tile_pool(name="up", bufs=2) as upl, \
                 tc.tile_pool(name="rp", bufs=2) as rpl, \
                 tc.tile_pool(name="sp", bufs=2, space="PSUM") as spl, \
                 tc.tile_pool(name="av", bufs=1, space="PSUM") as avl, \
                 tc.tile_pool(name="op", bufs=2, space="PSUM") as opl, \
                 tc.tile_pool(name="ot", bufs=3) as otl:

                def oproj_tile(st):
                    """Yield (emit-)closures: 8 matmul slots + finalizers."""
                    sts = slice(st * 128, (st + 1) * 128)
                    ps = [None, None]

                    def mk_mm(dh, ft):
                        def mm():
                            if ft == 0:
                                ps[dh] = opl.tile([128, 512], f32, tag="op",
                                                  name=f"ops{st}_{dh}")
                            nc.tensor.matmul(
                                ps[dh],
                                lhsT=attnT[:, ft, sts],
                                rhs=wo[:, ft, dh * 512:(dh + 1) * 512],
                                start=(ft == 0), stop=(ft == 3))
                            if ft == 3:
                                ot = otl.tile([128, 512], f32, tag="ot")
                                nc.vector.tensor_copy(ot, ps[dh])
                                nc.sync.dma_start(
                                    out=dout.ap()[sts,
                                                  dh * 512:(dh + 1) * 512],
                                    in_=ot)
                        return mm

                    return [mk_mm(dh, ft) for dh in range(2)
                            for ft in range(4)]

                def qproj_tile(qc, ft):
                    """8 matmul closures accumulating one QT ft-chunk."""
                    ps = [None]

                    def mk_mm(d):
                        def mm():
                            if d == 0:
                                ps[0] = opl.tile([128, 512], f32, tag="op",
                                                 name=f"qps{qc}_{ft}")
                            nc.tensor.matmul(
                                ps[0],
                                lhsT=wq[:, d, ft * 128:(ft + 1) * 128],
                                rhs=xq2[qc][:, d, :],
                                start=(d == 0), stop=(d == 7))
                            if d == 7:
                                nc.scalar.activation(
                                    QT[:, ft, qc * 512:(qc + 1) * 512],
                                    ps[0], IDn, bias=bq[:, ft:ft + 1])
                        return mm

                    return [mk_mm(d) for d in range(8)]

                def attn_head(qh, t, h, weave):
                    q0 = qh * 1024
                    p0 = h * 64
                    hh = 2 * t + h
                    E = epl.tile([128, KT_N, 1024], f16, tag="E")
                    av = avl.tile([65, 1024], f32, tag="av")

                    def scores_exp(kt):
                        s = spl.tile([128, 1024], f32, tag="s")
                        kts = slice(kt * 128, (kt + 1) * 128)
                        for c in range(2):
                            cs = slice(c * 512, (c + 1) * 512)
                            qs = slice(q0 + c * 512, q0 + (c + 1) * 512)
                            nc.tensor.matmul(
                                s[:, cs],
                                lhsT=KT[p0:p0 + 64, t, kts],
                                rhs=QT[p0:p0 + 64, t, qs],
                                start=True, stop=True)
                        nc.scalar.activation(
                            E[:, kt, :], s, EXP,
                            bias=mb[:, kt:kt + 1], scale=0.125)

                    # software pipeline: scores/exp run one tile ahead of
                    # the AV consumer; weave ops fill the exp-wait gap
                    scores_exp(0)
                    wv_i = 0
                    wv_n = len(weave)
                    for kt in range(KT_N):
                        if kt + 1 < KT_N:
                            scores_exp(kt + 1)
                        while wv_i < wv_n and wv_i * KT_N < (kt + 1) * wv_n:
                            weave[wv_i]()
                            wv_i += 1
                        for c in range(2):
                            cs = slice(c * 512, (c + 1) * 512)
                            nc.tensor.matmul(
                                av[:, cs],
                                lhsT=Vau[:, kt, hh, :],
                                rhs=E[:, kt, cs],
                                start=(kt == 0),
                                stop=(kt == KT_N - 1))

                    # normalize: attnT = av[0:64] * bcast(1/av[64])
                    dn = rpl.tile([1, 1024], f32, tag="dn")
                    nc.scalar.copy(dn, av[64:65, :])
                    rf = rpl.tile([1, 1024], f32, tag="rf")
                    nc.vector.reciprocal_approx_fast(out=rf, in_=dn)
                    r16 = rpl.tile([1, 1024], f16, tag="r16")
                    with nc.allow_low_precision(reason="fp16 recip"):
                        nc.vector.tensor_copy(r16, rf)
                    bcs = spl.tile([64, 1024], f32, tag="s")
                    for c in range(2):
                        cs = slice(c * 512, (c + 1) * 512)
                        nc.tensor.matmul(bcs[:, cs], lhsT=ones16[:, 0:64],
                                         rhs=r16[:, cs],
                                         start=True, stop=True)
                    u = upl.tile([64, 1024], f16, tag="u")
                    with nc.allow_low_precision(
                            reason="fp16 attn staging"):
                        nc.vector.tensor_copy(u, av[0:64, :])
                        nc.vector.tensor_mul(
                            attnT[p0:p0 + 64, t, q0:q0 + 1024],
                            u, bcs)

                qweave = [qproj_tile(qc, ft)
                          for qc in (2, 3) for ft in range(4)]
                for i, (t, h) in enumerate(
                        [(t, h) for t in range(4) for h in range(2)]):
                    attn_head(0, t, h, qweave[i])
                pending = []
                for i, (t, h) in enumerate(
                        [(t, h) for t in range(4) for h in range(2)]):
                    pending += oproj_tile(i)      # q-half 0 output tiles
                    attn_head(1, t, h, pending)
                    pending = []
                # tail: q-half 1 output tiles
                for st in range(8, 16):
                    for mm in oproj_tile(st):
                        mm()

    nc.compile()
    return nc


def _get_compiled(k_pad):
    if k_pad not in _COMPILED:
        _COMPILED[k_pad] = _build(k_pad)
    return _COMPILED[k_pad]


def _tile_pf(a, p=128):
    """[P*t, f...] -> contiguous [p, t, f...] partition-major tiling."""
    t = a.shape[0] // p
    return np.ascontiguousarray(
        a.reshape(t, p, *a.shape[1:]).swapaxes(0, 1))


def _prep_core_inputs(x, attention_mask, Wq, bq, Wk, bk, Wv, bv, Wo):
    """Host-side shard prep. Returns (in_maps, k_pad)."""
    x = np.asarray(x, np.float32)
    mask = np.asarray(attention_mask, bool)
    idxs = [np.nonzero(mask[b])[0] for b in range(BATCH)]
    ke_max = max(1, max(len(i) for i in idxs))
    k_pad = 384 * ((ke_max + 383) // 384)
    if k_pad > SEQ:
        k_pad = SEQ
    KC = 512 if k_pad % 512 == 0 else 384
    NKC = k_pad // KC
    KT_N = k_pad // 128

    ones16 = np.ones(128, F16)

    in_maps = []
    for b in range(BATCH):
        xT = x[b].T                                  # [D, S] view
        # xq: [qc, p, dt, 512]
        xq = np.ascontiguousarray(
            xT.reshape(8, 128, 4, 512).transpose(2, 1, 0, 3)).astype(F16)
        idx = idxs[b]
        ke = len(idx)
        if ke > k_pad:
            idx = idx[:k_pad]
            ke = k_pad
        xkT = np.zeros((D_MODEL, k_pad), np.float32)
        xkT[:, :ke] = x[b][idx].T
        # xk: [kc, p, dt, KC]
        xk = np.ascontiguousarray(
            xkT.reshape(8, 128, NKC, KC).transpose(2, 1, 0, 3)).astype(F16)
        maskb = np.zeros(k_pad, np.float32)
        maskb[ke:] = NEG
        mb_t = _tile_pf(maskb)                       # [128, KT_N]
        for g in range(2):
            fs = slice(g * FH, (g + 1) * FH)
            # Wv/bv padded with a ones column per head: the V-projection
            # matmul then produces [V_h | ones] directly (col = 0*x + 1.0).
            Wv_aug = np.zeros((D_MODEL, HPC * 65), np.float32)
            bv_aug = np.zeros(HPC * 65, np.float32)
            for h in range(HPC):
                Wv_aug[:, h * 65:h * 65 + 64] = Wv[:, g * FH + h * 64:
                                                   g * FH + (h + 1) * 64]
                bv_aug[h * 65:h * 65 + 64] = bv[g * FH + h * 64:
                                                g * FH + (h + 1) * 64]
                bv_aug[h * 65 + 64] = 1.0
            in_maps.append({
                "xq": xq,
                "xk": xk,
                "Wq": _tile_pf(np.asarray(Wq[:, fs], np.float32)).astype(F16),
                "Wk": _tile_pf(np.asarray(Wk[:, fs], np.float32)).astype(F16),
                "Wv": _tile_pf(Wv_aug).astype(F16),
                "Wo": _tile_pf(np.asarray(Wo[fs, :], np.float32)).astype(F16),
                "bcst": np.concatenate(
                    [_tile_pf(np.asarray(bq[fs], np.float32)),
                     _tile_pf(np.asarray(bk[fs], np.float32)),
                     mb_t], axis=1).astype(np.float32),
                "bv": bv_aug.astype(F16),
                "ones16": ones16,
            })
    return in_maps, k_pad


def kernel(x, attention_mask, Wq, bq, Wk, bk, Wv, bv, Wo, bo):
    global last_results
    from concourse.bass_utils import run_bass_kernel_spmd

    in_maps, k_pad = _prep_core_inputs(x, attention_mask, Wq, bq, Wk, bk,
                                       Wv, bv, Wo)
    nc = _get_compiled(k_pad)
    res = run_bass_kernel_spmd(nc, in_maps, core_ids=list(range(N_CORES)))
    last_results = res

    bo = np.asarray(bo, np.float32)
    out = np.empty((BATCH, SEQ, D_MODEL), np.float32)
    for b in range(BATCH):
        out[b] = res.results[2 * b]["out"] + res.results[2 * b + 1]["out"] + bo
    return out


# revision 16
# speedup vs baseline: 1.6102x; 1.3551x over previous
"""Multi-head attention kernel for 8 Trainium2 NeuronCores.

Problem: B=4, S=2048, D=1024, H=16, Dh=64 MHA with key-side boolean mask.

Sharding: core c handles (batch b = c//2, head-half g = c%2, 8 heads each).
QKV are column-parallel, the output projection is row-parallel (Megatron
style); the host sums the two partial output projections per batch and adds
the output bias.

Host-side preprocessing (pure data marshalling):
  - All inputs are pre-tiled into DMA-native layouts (partition-major,
    contiguous per partition).
  - x is transposed per batch (the PE contracts over the partition dim).
  - Keys with mask=False contribute exactly zero after softmax, so the host
    gathers only the unmasked keys (padded to a multiple of 384 with zero
    rows whose exp-bias is -1e30 => exp == 0 exactly).
  - All matmul operands are fp16 (same PE throughput as bf16 on TRN2 but
    8x lower quantization noise; attention averages ~1e3 near-uniform keys
    so per-element noise in E/V passes straight to the output).

On-core dataflow (all matmuls fp16, PSUM accumulation fp32):
  xT --(Wk)--> KT[f,k]             bias fused in the ScalarE PSUM->SBUF copy
  xT --(Wv)--> Vau[k, h, 65]       (aug ones col -> softmax denominator)
  xT --(Wq)--> QT[f,q]
  scores[k,q] = KT_h^T x QT_h      64-deep contraction at base partition
                                   0/64 (cost is column-count bound)
  E = exp(scores*0.125 + maskbias[k])   one ScalarE pass per key tile,
                                   written to SBUF as fp16; ScalarE does
                                   nothing else during attention
  av[65,q] += Vau_kt^T x E_kt      accumulated over key tiles in PSUM
  attnT[f,q] = av[0:64] * bcast(1/av[64])  (ones-matmul broadcast + DVE)
  out[s,D] = attnT^T x Wo          (partial; host adds pair + bo)
"""

import os
import numpy as np

os.environ.setdefault("MYCRO_LOCAL_CACHE", "1")

D_MODEL = 1024
N_HEADS = 16
D_HEAD = 64
BATCH = 4
SEQ = 2048
N_CORES = 8
FH = 512          # features per core (8 heads x 64)
HPC = 8           # heads per core
NEG = -1.0e30     # additive bias for padded/masked keys; exp -> 0 exactly

F16 = np.float16

_COMPILED = {}    # k_pad -> nc
last_results = None  # BassKernelResults of the most recent run (for test.py)


def _build(k_pad):
    """Emit + compile the per-core bass kernel for a given padded key count."""
    import concourse.bacc as bacc
    import concourse.bass as bass
    import concourse.tile as tile
    from concourse import mybir

    f32 = mybir.dt.float32
    f32r = mybir.dt.float32r
    f16 = mybir.dt.float16

    KT_N = k_pad // 128                     # number of 128-key tiles
    KC = 512 if k_pad % 512 == 0 else 384   # key-side chunk
    assert k_pad % KC == 0 and KC % 128 == 0
    NKC = k_pad // KC
    HW = HPC * 65   # augmented V width (520)

    nc = bacc.Bacc("TRN2", target_bir_lowering=False, debug=False,
                   num_devices=N_CORES)

    # all pre-tiled on host into DMA-native layouts
    dxq = nc.dram_tensor("xq", [4, 128, 8, 512], f16, kind="ExternalInput")
    dxk = nc.dram_tensor("xk", [NKC, 128, 8, KC], f16, kind="ExternalInput")
    dWq = nc.dram_tensor("Wq", [128, 8, FH], f16, kind="ExternalInput")
    dWk = nc.dram_tensor("Wk", [128, 8, FH], f16, kind="ExternalInput")
    dWv = nc.dram_tensor("Wv", [128, 8, HW], f16, kind="ExternalInput")
    dWo = nc.dram_tensor("Wo", [128, 4, D_MODEL], f16, kind="ExternalInput")
    dbc = nc.dram_tensor("bcst", [128, 8 + KT_N], f32, kind="ExternalInput")
    dbv = nc.dram_tensor("bv", [HW], f16, kind="ExternalInput")
    dc16 = nc.dram_tensor("ones16", [128], f16, kind="ExternalInput")
    drs = nc.dram_tensor("rscratch", [16, 1024], f16, kind="Internal")
    dout = nc.dram_tensor("out", [SEQ, D_MODEL], f32, kind="ExternalOutput")

    EXP = mybir.ActivationFunctionType.Exp
    IDn = mybir.ActivationFunctionType.Identity

    with tile.TileContext(nc) as tc:
        with tc.tile_pool(name="persist", bufs=1) as pers:
            # ---- constants in SBUF ----
            bc = pers.tile([128, 8 + KT_N], f32, tag="bcst")
            nc.sync.dma_start(out=bc, in_=dbc.ap())
            bq = bc[:, 0:4]
            bk = bc[:, 4:8]
            mb = bc[:, 8:8 + KT_N]
            bv_row = pers.tile([1, HW], f16, tag="bvr")
            nc.sync.dma_start(out=bv_row, in_=dbv.ap()[None, :])
            ones16 = pers.tile([1, 128], f16, tag="ones16")
            nc.sync.dma_start(out=ones16, in_=dc16.ap()[None, :])

            # ---- persistent activations ----
            QT = pers.tile([128, 4, SEQ], f16, tag="QT")         # [f, q]
            KT = pers.tile([128, 4, k_pad], f16, tag="KT")       # [f, k]
            Vau = pers.tile([128, KT_N, HPC, 65], f16, tag="Vau")
            attnT = pers.tile([128, 4, SEQ], f16, tag="attnT")   # [f, q]
            wo = pers.tile([128, 4, D_MODEL], f16, tag="wo")

            # ================= projections =================
            wq = pers.tile([128, 8, FH], f16, tag="wq")
            nc.sync.dma_start(out=wq, in_=dWq.ap())
            xq2a = pers.tile([128, 8, 512], f16, tag="xq2a")
            nc.sync.dma_start(out=xq2a, in_=dxq.ap()[2])
            xq2b = pers.tile([128, 8, 512], f16, tag="xq2b")
            nc.sync.dma_start(out=xq2b, in_=dxq.ap()[3])
            xq2 = {2: xq2a, 3: xq2b}
            ppool_cm = tc.tile_pool(name="pp", bufs=4, space="PSUM")
            ppool = ppool_cm.__enter__()

            # ----- K side (KT, V) -----
            with tc.tile_pool(name="wtk", bufs=1) as wtk, \
                 tc.tile_pool(name="xk", bufs=2) as xkp:
                pk = ppool
                wk = wtk.tile([128, 8, FH], f16, tag="wk")
                nc.sync.dma_start(out=wk, in_=dWk.ap())
                wv = wtk.tile([128, 8, HW], f16, tag="wv")
                nc.sync.dma_start(out=wv, in_=dWv.ap())
                for kc in range(NKC):
                    xk_t = xkp.tile([128, 8, KC], f16, tag="xk")
                    nc.sync.dma_start(out=xk_t, in_=dxk.ap()[kc])
                    for ft in range(4):
                        ps = pk.tile([128, KC], f32, tag="pk")
                        for d in range(8):
                            nc.tensor.matmul(
                                ps,
                                lhsT=wk[:, d, ft * 128:(ft + 1) * 128],
                                rhs=xk_t[:, d, :],
                                start=(d == 0), stop=(d == 7))
                        ks = slice(kc * KC, (kc + 1) * KC)
                        nc.scalar.activation(KT[:, ft, ks], ps, IDn,
                                             bias=bk[:, ft:ft + 1])
                    for kb in range(KC // 128):
                        kg = kc * (KC // 128) + kb
                        ps = pk.tile([128, HW], f32, tag="pk")
                        for d in range(8):
                            lt = xk_t[:, d, kb * 128:(kb + 1) * 128]
                            nc.tensor.matmul(
                                ps[:, 0:512], lhsT=lt,
                                rhs=wv[:, d, 0:512],
                                start=(d == 0), stop=False)
                            nc.tensor.matmul(
                                ps[:, 512:520], lhsT=lt,
                                rhs=wv[:, d, 512:520],
                                start=(d == 0), stop=False)
                        nc.tensor.matmul(ps[:, 0:512], lhsT=ones16,
                                         rhs=bv_row[:, 0:512],
                                         start=False, stop=True)
                        nc.tensor.matmul(ps[:, 512:520], lhsT=ones16,
                                         rhs=bv_row[:, 512:520],
                                         start=False, stop=True)
                        nc.scalar.copy(Vau[:, kg, :, :], ps)

            # ----- Q side (QT): qc 0,1 here; qc 2,3 woven into the
            # qh=0 attention loop (their queries are only read in qh=1)
            with tc.tile_pool(name="xq", bufs=2) as xqp:
                pq = ppool
                for qc in range(2):
                    xq_t = xqp.tile([128, 8, 512], f16, tag="xq")
                    nc.sync.dma_start(out=xq_t, in_=dxq.ap()[qc])
                    for ft in range(4):
                        ps = pq.tile([128, 512], f32, tag="pk")
                        for d in range(8):
                            nc.tensor.matmul(
                                ps,
                                lhsT=wq[:, d, ft * 128:(ft + 1) * 128],
                                rhs=xq_t[:, d, :],
                                start=(d == 0), stop=(d == 7))
                        nc.scalar.activation(QT[:, ft, qc * 512:(qc + 1) * 512],
                                             ps, IDn, bias=bq[:, ft:ft + 1])

            ppool_cm.__exit__(None, None, None)
            nc.sync.dma_start(out=wo, in_=dWo.ap())

            # ================= attention core =================
            # Per (qh, t, h): KT_N score tiles [128k, 1024q] through a
            # double-buffered PSUM pool; exp each tile straight to fp16 E
            # in SBUF; AV accumulates over key tiles in PSUM. ScalarE does
            # only exp here. O-projection matmuls for the finished query
            # half are woven one-per-key-tile-slot into the other half's
            # attention loop, keeping the PE busy (full p-state) while it
            # would otherwise wait on ScalarE.
            with tc.tile_pool(name="ep", bufs=2) as epl, \
                 tc.tile_pool(name="up", bufs=2) as upl, \
                 tc.tile_pool(name="rp", bufs=2) as rpl, \
                 tc.tile_pool(name="sp", bufs=2, space="PSUM") as spl, \
                 tc.tile_pool(name="av", bufs=1, space="PSUM") as avl, \
                 tc.tile_pool(name="op", bufs=2, space="PSUM") as opl, \
                 tc.tile_pool(name="ot", bufs=3) as otl:

                def oproj_tile(st):
                    """Yield (emit-)closures: 8 matmul slots + finalizers."""
                    sts = slice(st * 128, (st + 1) * 128)
                    ps = [None, None]

                    def mk_mm(dh, ft):
                        def mm():
                            if ft == 0:
                                ps[dh] = opl.tile([128, 512], f32, tag="op",
                                                  name=f"ops{st}_{dh}")
                            nc.tensor.matmul(
                                ps[dh],
                                lhsT=attnT[:, ft, sts],
                                rhs=wo[:, ft, dh * 512:(dh + 1) * 512],
                                start=(ft == 0), stop=(ft == 3))
                            if ft == 3:
                                ot = otl.tile([128, 512], f32, tag="ot")
                                nc.vector.tensor_copy(ot, ps[dh])
                                nc.sync.dma_start(
                                    out=dout.ap()[sts,
                                                  dh * 512:(dh + 1) * 512],
                                    in_=ot)
                        return mm

                    return [mk_mm(dh, ft) for dh in range(2)
                            for ft in range(4)]

                def qproj_tile(qc, ft):
                    """8 matmul closures accumulating one QT ft-chunk."""
                    ps = [None]

                    def mk_mm(d):
                        def mm():
                            if d == 0:
                                ps[0] = opl.tile([128, 512], f32, tag="op",
                                                 name=f"qps{qc}_{ft}")
                            nc.tensor.matmul(
                                ps[0],
                                lhsT=wq[:, d, ft * 128:(ft + 1) * 128],
                                rhs=xq2[qc][:, d, :],
                                start=(d == 0), stop=(d == 7))
                            if d == 7:
                                nc.scalar.activation(
                                    QT[:, ft, qc * 512:(qc + 1) * 512],
                                    ps[0], IDn, bias=bq[:, ft:ft + 1])
                        return mm

                    return [mk_mm(d) for d in range(8)]

                def attn_head(qh, t, h, weave):
                    q0 = qh * 1024
                    p0 = h * 64
                    hh = 2 * t + h
                    E = epl.tile([128, KT_N, 1024], f16, tag="E")
                    av = avl.tile([65, 1024], f32, tag="av")

                    def scores_exp(kt):
                        s = spl.tile([128, 1024], f32, tag="s")
                        kts = slice(kt * 128, (kt + 1) * 128)
                        for c in range(2):
                            cs = slice(c * 512, (c + 1) * 512)
                            qs = slice(q0 + c * 512, q0 + (c + 1) * 512)
                            nc.tensor.matmul(
                                s[:, cs],
                                lhsT=KT[p0:p0 + 64, t, kts],
                                rhs=QT[p0:p0 + 64, t, qs],
                                start=True, stop=True)
                        nc.scalar.activation(
                            E[:, kt, :], s, EXP,
                            bias=mb[:, kt:kt + 1], scale=0.125)

                    # software pipeline: scores/exp run one tile ahead of
                    # the AV consumer; weave ops fill the exp-wait gap
                    scores_exp(0)
                    wv_i = 0
                    wv_n = len(weave)
                    for kt in range(KT_N):
                        if kt + 1 < KT_N:
                            scores_exp(kt + 1)
                        while wv_i < wv_n and wv_i * KT_N < (kt + 1) * wv_n:
                            weave[wv_i]()
                            wv_i += 1
                        for c in range(2):
                            cs = slice(c * 512, (c + 1) * 512)
                            nc.tensor.matmul(
                                av[:, cs],
                                lhsT=Vau[:, kt, hh, :],
                                rhs=E[:, kt, cs],
                                start=(kt == 0),
                                stop=(kt == KT_N - 1))

                    # normalize: attnT = av[0:64] * bcast(1/av[64])
                    dn = rpl.tile([1, 1024], f32, tag="dn")
                    nc.scalar.copy(dn, av[64:65, :])
                    rf = rpl.tile([1, 1024], f32, tag="rf")
                    nc.vector.reciprocal_approx_fast(out=rf, in_=dn)
                    r16 = rpl.tile([1, 1024], f16, tag="r16")
                    with nc.allow_low_precision(reason="fp16 recip"):
                        nc.vector.tensor_copy(r16, rf)
                    # broadcast 1/den across 64 partitions via a DRAM
                    # bounce with a stride-0-partition read (keeps the
                    # score PSUM pool free of normalize traffic, so the
                    # exp pipeline flows across head boundaries)
                    ri = (qh * 8 + t * 2 + h)
                    nc.sync.dma_start(out=drs.ap()[ri][None, :], in_=r16)
                    bc16 = upl.tile([64, 1024], f16, tag="bc")
                    nc.sync.dma_start(
                        out=bc16,
                        in_=bass.AP(tensor=drs.ap().tensor,
                                    offset=ri * 1024,
                                    ap=[[0, 64], [1, 1024]]))
                    u = upl.tile([64, 1024], f16, tag="u")
                    with nc.allow_low_precision(
                            reason="fp16 attn staging"):
                        nc.vector.tensor_copy(u, av[0:64, :])
                        nc.vector.tensor_mul(
                            attnT[p0:p0 + 64, t, q0:q0 + 1024],
                            u, bc16)

                qweave = [qproj_tile(qc, ft)
                          for qc in (2, 3) for ft in range(4)]
                for i, (t, h) in enumerate(
                        [(t, h) for t in range(4) for h in range(2)]):
                    attn_head(0, t, h, qweave[i])
                pending = []
                for i, (t, h) in enumerate(
                        [(t, h) for t in range(4) for h in range(2)]):
                    pending += oproj_tile(i)      # q-half 0 output tiles
                    attn_head(1, t, h, pending)
                    pending = []
                # tail: q-half 1 output tiles
                for st in range(8, 16):
                    for mm in oproj_tile(st):
                        mm()

    nc.compile()
    return nc


def _get_compiled(k_pad):
    if k_pad not in _COMPILED:
        _COMPILED[k_pad] = _build(k_pad)
    return _COMPILED[k_pad]


def _tile_pf(a, p=128):
    """[P*t, f...] -> contiguous [p, t, f...] partition-major tiling."""
    t = a.shape[0] // p
    return np.ascontiguousarray(
        a.reshape(t, p, *a.shape[1:]).swapaxes(0, 1))


def _prep_core_inputs(x, attention_mask, Wq, bq, Wk, bk, Wv, bv, Wo):
    """Host-side shard prep. Returns (in_maps, k_pad)."""
    x = np.asarray(x, np.float32)
    mask = np.asarray(attention_mask, bool)
    idxs = [np.nonzero(mask[b])[0] for b in range(BATCH)]
    ke_max = max(1, max(len(i) for i in idxs))
    k_pad = 384 * ((ke_max + 383) // 384)
    if k_pad > SEQ:
        k_pad = SEQ
    KC = 512 if k_pad % 512 == 0 else 384
    NKC = k_pad // KC
    KT_N = k_pad // 128

    ones16 = np.ones(128, F16)

    in_maps = []
    for b in range(BATCH):
        xT = x[b].T                                  # [D, S] view
        # xq: [qc, p, dt, 512]
        xq = np.ascontiguousarray(
            xT.reshape(8, 128, 4, 512).transpose(2, 1, 0, 3)).astype(F16)
        idx = idxs[b]
        ke = len(idx)
        if ke > k_pad:
            idx = idx[:k_pad]
            ke = k_pad
        xkT = np.zeros((D_MODEL, k_pad), np.float32)
        xkT[:, :ke] = x[b][idx].T
        # xk: [kc, p, dt, KC]
        xk = np.ascontiguousarray(
            xkT.reshape(8, 128, NKC, KC).transpose(2, 1, 0, 3)).astype(F16)
        maskb = np.zeros(k_pad, np.float32)
        maskb[ke:] = NEG
        mb_t = _tile_pf(maskb)                       # [128, KT_N]
        for g in range(2):
            fs = slice(g * FH, (g + 1) * FH)
            # Wv/bv padded with a ones column per head: the V-projection
            # matmul then produces [V_h | ones] directly (col = 0*x + 1.0).
            Wv_aug = np.zeros((D_MODEL, HPC * 65), np.float32)
            bv_aug = np.zeros(HPC * 65, np.float32)
            for h in range(HPC):
                Wv_aug[:, h * 65:h * 65 + 64] = Wv[:, g * FH + h * 64:
                                                   g * FH + (h + 1) * 64]
                bv_aug[h * 65:h * 65 + 64] = bv[g * FH + h * 64:
                                                g * FH + (h + 1) * 64]
                bv_aug[h * 65 + 64] = 1.0
            in_maps.append({
                "xq": xq,
                "xk": xk,
                "Wq": _tile_pf(np.asarray(Wq[:, fs], np.float32)).astype(F16),
                "Wk": _tile_pf(np.asarray(Wk[:, fs], np.float32)).astype(F16),
                "Wv": _tile_pf(Wv_aug).astype(F16),
                "Wo": _tile_pf(np.asarray(Wo[fs, :], np.float32)).astype(F16),
                "bcst": np.concatenate(
                    [_tile_pf(np.asarray(bq[fs], np.float32)),
                     _tile_pf(np.asarray(bk[fs], np.float32)),
                     mb_t], axis=1).astype(np.float32),
                "bv": bv_aug.astype(F16),
                "ones16": ones16,
            })
    return in_maps, k_pad


def kernel(x, attention_mask, Wq, bq, Wk, bk, Wv, bv, Wo, bo):
    global last_results
    from concourse.bass_utils import run_bass_kernel_spmd

    in_maps, k_pad = _prep_core_inputs(x, attention_mask, Wq, bq, Wk, bk,
                                       Wv, bv, Wo)
    nc = _get_compiled(k_pad)
    res = run_bass_kernel_spmd(nc, in_maps, core_ids=list(range(N_CORES)))
    last_results = res

    bo = np.asarray(bo, np.float32)
    out = np.empty((BATCH, SEQ, D_MODEL), np.float32)
    for b in range(BATCH):
        out[b] = res.results[2 * b]["out"] + res.results[2 * b + 1]["out"] + bo
    return out


# revision 17
# speedup vs baseline: 1.6889x; 1.0489x over previous
"""Multi-head attention kernel for 8 Trainium2 NeuronCores.

Problem: B=4, S=2048, D=1024, H=16, Dh=64 MHA with key-side boolean mask.

Sharding: core c handles (batch b = c//2, head-half g = c%2, 8 heads each).
QKV are column-parallel, the output projection is row-parallel (Megatron
style); the host sums the two partial output projections per batch and adds
the output bias.

Host-side preprocessing (pure data marshalling):
  - All inputs are pre-tiled into DMA-native layouts (partition-major,
    contiguous per partition).
  - x is transposed per batch (the PE contracts over the partition dim).
  - Keys with mask=False contribute exactly zero after softmax, so the host
    gathers only the unmasked keys (padded to a multiple of 384 with zero
    rows whose exp-bias is -1e30 => exp == 0 exactly).
  - All matmul operands are fp16 (same PE throughput as bf16 on TRN2 but
    8x lower quantization noise; attention averages ~1e3 near-uniform keys
    so per-element noise in E/V passes straight to the output).

On-core dataflow (all matmuls fp16, PSUM accumulation fp32):
  xT --(Wk)--> KT[f,k]             bias fused in the ScalarE PSUM->SBUF copy
  xT --(Wv)--> Vau[k, h, 65]       (aug ones col -> softmax denominator)
  xT --(Wq)--> QT[f,q]
  scores[k,q] = KT_h^T x QT_h      64-deep contraction at base partition
                                   0/64 (cost is column-count bound)
  E = exp(scores*0.125 + maskbias[k])   one ScalarE pass per key tile,
                                   written to SBUF as fp16; ScalarE does
                                   nothing else during attention
  av[65,q] += Vau_kt^T x E_kt      accumulated over key tiles in PSUM
  attnT[f,q] = av[0:64] * bcast(1/av[64])  (ones-matmul broadcast + DVE)
  out[s,D] = attnT^T x Wo          (partial; host adds pair + bo)
"""

import os
import numpy as np

os.environ.setdefault("MYCRO_LOCAL_CACHE", "1")

D_MODEL = 1024
N_HEADS = 16
D_HEAD = 64
BATCH = 4
SEQ = 2048
N_CORES = 8
FH = 512          # features per core (8 heads x 64)
HPC = 8           # heads per core
NEG = -1.0e30     # additive bias for padded/masked keys; exp -> 0 exactly

F16 = np.float16

_COMPILED = {}    # k_pad -> nc
last_results = None  # BassKernelResults of the most recent run (for test.py)


def _build(k_pad):
    """Emit + compile the per-core bass kernel for a given padded key count."""
    import concourse.bacc as bacc
    import concourse.bass as bass
    import concourse.tile as tile
    from concourse import mybir

    f32 = mybir.dt.float32
    f32r = mybir.dt.float32r
    f16 = mybir.dt.float16

    KT_N = k_pad // 128                     # number of 128-key tiles
    KC = 512 if k_pad % 512 == 0 else 384   # key-side chunk
    assert k_pad % KC == 0 and KC % 128 == 0
    NKC = k_pad // KC
    HW = HPC * 65   # augmented V width (520)

    nc = bacc.Bacc("TRN2", target_bir_lowering=False, debug=False,
                   num_devices=N_CORES)

    # all pre-tiled on host into DMA-native layouts
    dxq = nc.dram_tensor("xq", [4, 128, 8, 512], f16, kind="ExternalInput")
    dxk = nc.dram_tensor("xk", [NKC, 128, 8, KC], f16, kind="ExternalInput")
    dWq = nc.dram_tensor("Wq", [128, 8, FH], f16, kind="ExternalInput")
    dWk = nc.dram_tensor("Wk", [128, 8, FH], f16, kind="ExternalInput")
    dWv = nc.dram_tensor("Wv", [128, 8, HW], f16, kind="ExternalInput")
    dWo = nc.dram_tensor("Wo", [128, 4, D_MODEL], f16, kind="ExternalInput")
    dbc = nc.dram_tensor("bcst", [128, 8 + KT_N], f32, kind="ExternalInput")
    dbv = nc.dram_tensor("bv", [HW], f16, kind="ExternalInput")
    dc16 = nc.dram_tensor("ones16", [128], f16, kind="ExternalInput")
    drs = nc.dram_tensor("rscratch", [16, 1024], f16, kind="Internal")
    dout = nc.dram_tensor("out", [SEQ, D_MODEL], f32, kind="ExternalOutput")

    EXP = mybir.ActivationFunctionType.Exp
    IDn = mybir.ActivationFunctionType.Identity

    with tile.TileContext(nc) as tc:
        with tc.tile_pool(name="persist", bufs=1) as pers:
            # ---- constants in SBUF ----
            bc = pers.tile([128, 8 + KT_N], f32, tag="bcst")
            nc.sync.dma_start(out=bc, in_=dbc.ap())
            bq = bc[:, 0:4]
            bk = bc[:, 4:8]
            mb = bc[:, 8:8 + KT_N]
            bv_row = pers.tile([1, HW], f16, tag="bvr")
            nc.sync.dma_start(out=bv_row, in_=dbv.ap()[None, :])
            ones16 = pers.tile([1, 128], f16, tag="ones16")
            nc.sync.dma_start(out=ones16, in_=dc16.ap()[None, :])

            # ---- persistent activations ----
            QT = pers.tile([128, 4, SEQ], f16, tag="QT")         # [f, q]
            KT = pers.tile([128, 4, k_pad], f16, tag="KT")       # [f, k]
            Vau = pers.tile([128, KT_N, HPC, 65], f16, tag="Vau")
            attnT = pers.tile([128, 4, SEQ], f16, tag="attnT")   # [f, q]
            wo = pers.tile([128, 4, D_MODEL], f16, tag="wo")

            # ================= projections =================
            # (wq/xq DMAs are emitted after the K-side DMAs so the first
            # K-projection matmul isn't stuck behind 3MB of Q-side input)
            wq = pers.tile([128, 8, FH], f16, tag="wq")
            xq2a = pers.tile([128, 8, 512], f16, tag="xq2a")
            xq2b = pers.tile([128, 8, 512], f16, tag="xq2b")
            xq2 = {2: xq2a, 3: xq2b}
            ppool_cm = tc.tile_pool(name="pp", bufs=4, space="PSUM")
            ppool = ppool_cm.__enter__()

            # ----- K side (KT, V) -----
            with tc.tile_pool(name="wtk", bufs=1) as wtk, \
                 tc.tile_pool(name="xk", bufs=2) as xkp:
                pk = ppool
                wk = wtk.tile([128, 8, FH], f16, tag="wk")
                nc.sync.dma_start(out=wk, in_=dWk.ap())
                wv = wtk.tile([128, 8, HW], f16, tag="wv")
                nc.sync.dma_start(out=wv, in_=dWv.ap())
                for kc in range(NKC):
                    xk_t = xkp.tile([128, 8, KC], f16, tag="xk")
                    nc.sync.dma_start(out=xk_t, in_=dxk.ap()[kc])
                    for ft in range(4):
                        ps = pk.tile([128, KC], f32, tag="pk")
                        for d in range(8):
                            nc.tensor.matmul(
                                ps,
                                lhsT=wk[:, d, ft * 128:(ft + 1) * 128],
                                rhs=xk_t[:, d, :],
                                start=(d == 0), stop=(d == 7))
                        ks = slice(kc * KC, (kc + 1) * KC)
                        nc.scalar.activation(KT[:, ft, ks], ps, IDn,
                                             bias=bk[:, ft:ft + 1])
                    for kb in range(KC // 128):
                        kg = kc * (KC // 128) + kb
                        ps = pk.tile([128, HW], f32, tag="pk")
                        for d in range(8):
                            lt = xk_t[:, d, kb * 128:(kb + 1) * 128]
                            nc.tensor.matmul(
                                ps[:, 0:512], lhsT=lt,
                                rhs=wv[:, d, 0:512],
                                start=(d == 0), stop=False)
                            nc.tensor.matmul(
                                ps[:, 512:520], lhsT=lt,
                                rhs=wv[:, d, 512:520],
                                start=(d == 0), stop=False)
                        nc.tensor.matmul(ps[:, 0:512], lhsT=ones16,
                                         rhs=bv_row[:, 0:512],
                                         start=False, stop=True)
                        nc.tensor.matmul(ps[:, 512:520], lhsT=ones16,
                                         rhs=bv_row[:, 512:520],
                                         start=False, stop=True)
                        nc.scalar.copy(Vau[:, kg, :, :], ps)

            # ----- Q side (QT): qc 0,1 here; qc 2,3 woven into the
            # qh=0 attention loop (their queries are only read in qh=1)
            nc.sync.dma_start(out=wq, in_=dWq.ap())
            nc.sync.dma_start(out=xq2a, in_=dxq.ap()[2])
            nc.sync.dma_start(out=xq2b, in_=dxq.ap()[3])
            with tc.tile_pool(name="xq", bufs=2) as xqp:
                pq = ppool
                for qc in range(2):
                    xq_t = xqp.tile([128, 8, 512], f16, tag="xq")
                    nc.sync.dma_start(out=xq_t, in_=dxq.ap()[qc])
                    for ft in range(4):
                        ps = pq.tile([128, 512], f32, tag="pk")
                        for d in range(8):
                            nc.tensor.matmul(
                                ps,
                                lhsT=wq[:, d, ft * 128:(ft + 1) * 128],
                                rhs=xq_t[:, d, :],
                                start=(d == 0), stop=(d == 7))
                        nc.scalar.activation(QT[:, ft, qc * 512:(qc + 1) * 512],
                                             ps, IDn, bias=bq[:, ft:ft + 1])

            ppool_cm.__exit__(None, None, None)
            nc.sync.dma_start(out=wo, in_=dWo.ap())

            # ================= attention core =================
            # Per (qh, t, h): KT_N score tiles [128k, 1024q] through a
            # double-buffered PSUM pool; exp each tile straight to fp16 E
            # in SBUF; AV accumulates over key tiles in PSUM. ScalarE does
            # only exp here. O-projection matmuls for the finished query
            # half are woven one-per-key-tile-slot into the other half's
            # attention loop, keeping the PE busy (full p-state) while it
            # would otherwise wait on ScalarE.
            with tc.tile_pool(name="ep", bufs=2) as epl, \
                 tc.tile_pool(name="up", bufs=2) as upl, \
                 tc.tile_pool(name="rp", bufs=2) as rpl, \
                 tc.tile_pool(name="sp", bufs=2, space="PSUM") as spl, \
                 tc.tile_pool(name="av", bufs=1, space="PSUM") as avl, \
                 tc.tile_pool(name="op", bufs=2, space="PSUM") as opl, \
                 tc.tile_pool(name="ot", bufs=3) as otl:

                def oproj_tile(st):
                    """Yield (emit-)closures: 8 matmul slots + finalizers."""
                    sts = slice(st * 128, (st + 1) * 128)
                    ps = [None, None]

                    def mk_mm(dh, ft):
                        def mm():
                            if ft == 0:
                                ps[dh] = opl.tile([128, 512], f32, tag="op",
                                                  name=f"ops{st}_{dh}")
                            nc.tensor.matmul(
                                ps[dh],
                                lhsT=attnT[:, ft, sts],
                                rhs=wo[:, ft, dh * 512:(dh + 1) * 512],
                                start=(ft == 0), stop=(ft == 3))
                            if ft == 3:
                                ot = otl.tile([128, 512], f32, tag="ot")
                                nc.vector.tensor_copy(ot, ps[dh])
                                nc.sync.dma_start(
                                    out=dout.ap()[sts,
                                                  dh * 512:(dh + 1) * 512],
                                    in_=ot)
                        return mm

                    return [mk_mm(dh, ft) for dh in range(2)
                            for ft in range(4)]

                def qproj_tile(qc, ft):
                    """8 matmul closures accumulating one QT ft-chunk."""
                    ps = [None]

                    def mk_mm(d):
                        def mm():
                            if d == 0:
                                ps[0] = opl.tile([128, 512], f32, tag="op",
                                                 name=f"qps{qc}_{ft}")
                            nc.tensor.matmul(
                                ps[0],
                                lhsT=wq[:, d, ft * 128:(ft + 1) * 128],
                                rhs=xq2[qc][:, d, :],
                                start=(d == 0), stop=(d == 7))
                            if d == 7:
                                nc.scalar.activation(
                                    QT[:, ft, qc * 512:(qc + 1) * 512],
                                    ps[0], IDn, bias=bq[:, ft:ft + 1])
                        return mm

                    return [mk_mm(d) for d in range(8)]

                def attn_head(qh, t, h, weave):
                    q0 = qh * 1024
                    p0 = h * 64
                    hh = 2 * t + h
                    E = epl.tile([128, KT_N, 1024], f16, tag="E")
                    av = avl.tile([65, 1024], f32, tag="av")

                    def scores_exp(kt):
                        s = spl.tile([128, 1024], f32, tag="s")
                        kts = slice(kt * 128, (kt + 1) * 128)
                        for c in range(2):
                            cs = slice(c * 512, (c + 1) * 512)
                            qs = slice(q0 + c * 512, q0 + (c + 1) * 512)
                            nc.tensor.matmul(
                                s[:, cs],
                                lhsT=KT[p0:p0 + 64, t, kts],
                                rhs=QT[p0:p0 + 64, t, qs],
                                start=True, stop=True)
                        nc.scalar.activation(
                            E[:, kt, :], s, EXP,
                            bias=mb[:, kt:kt + 1], scale=0.125)

                    # software pipeline: scores/exp run one tile ahead of
                    # the AV consumer; weave ops fill the exp-wait gap
                    scores_exp(0)
                    wv_i = 0
                    wv_n = len(weave)
                    for kt in range(KT_N):
                        if kt + 1 < KT_N:
                            scores_exp(kt + 1)
                        while wv_i < wv_n and wv_i * KT_N < (kt + 1) * wv_n:
                            weave[wv_i]()
                            wv_i += 1
                        for c in range(2):
                            cs = slice(c * 512, (c + 1) * 512)
                            nc.tensor.matmul(
                                av[:, cs],
                                lhsT=Vau[:, kt, hh, :],
                                rhs=E[:, kt, cs],
                                start=(kt == 0),
                                stop=(kt == KT_N - 1))

                    # normalize: attnT = av[0:64] * bcast(1/av[64]).
                    # u-copy is emitted first so the av PSUM tile frees
                    # for the next head before the recip chain drains.
                    dn = rpl.tile([1, 1024], f32, tag="dn")
                    nc.scalar.copy(dn, av[64:65, :])
                    u = upl.tile([64, 1024], f16, tag="u")
                    with nc.allow_low_precision(reason="fp16 attn staging"):
                        nc.vector.tensor_copy(u, av[0:64, :])
                    rf = rpl.tile([1, 1024], f32, tag="rf")
                    nc.vector.reciprocal_approx_fast(out=rf, in_=dn)
                    r16 = rpl.tile([1, 1024], f16, tag="r16")
                    with nc.allow_low_precision(reason="fp16 recip"):
                        nc.vector.tensor_copy(r16, rf)
                    # broadcast 1/den across 64 partitions via a DRAM
                    # bounce with a stride-0-partition read (keeps the
                    # score PSUM pool free of normalize traffic, so the
                    # exp pipeline flows across head boundaries)
                    ri = (qh * 8 + t * 2 + h)
                    nc.sync.dma_start(out=drs.ap()[ri][None, :], in_=r16)
                    bc16 = upl.tile([64, 1024], f16, tag="bc")
                    nc.sync.dma_start(
                        out=bc16,
                        in_=bass.AP(tensor=drs.ap().tensor,
                                    offset=ri * 1024,
                                    ap=[[0, 64], [1, 1024]]))
                    with nc.allow_low_precision(
                            reason="fp16 attn staging"):
                        nc.vector.tensor_mul(
                            attnT[p0:p0 + 64, t, q0:q0 + 1024],
                            u, bc16)

                qweave = [qproj_tile(qc, ft)
                          for qc in (2, 3) for ft in range(4)]
                for i, (t, h) in enumerate(
                        [(t, h) for t in range(4) for h in range(2)]):
                    attn_head(0, t, h, qweave[i])
                pending = []
                for i, (t, h) in enumerate(
                        [(t, h) for t in range(4) for h in range(2)]):
                    pending += oproj_tile(i)      # q-half 0 output tiles
                    attn_head(1, t, h, pending)
                    pending = []
                # tail: q-half 1 output tiles
                for st in range(8, 16):
                    for mm in oproj_tile(st):
                        mm()

    nc.compile()
    return nc


def _get_compiled(k_pad):
    if k_pad not in _COMPILED:
        _COMPILED[k_pad] = _build(k_pad)
    return _COMPILED[k_pad]


def _tile_pf(a, p=128):
    """[P*t, f...] -> contiguous [p, t, f...] partition-major tiling."""
    t = a.shape[0] // p
    return np.ascontiguousarray(
        a.reshape(t, p, *a.shape[1:]).swapaxes(0, 1))


def _prep_core_inputs(x, attention_mask, Wq, bq, Wk, bk, Wv, bv, Wo):
    """Host-side shard prep. Returns (in_maps, k_pad)."""
    x = np.asarray(x, np.float32)
    mask = np.asarray(attention_mask, bool)
    idxs = [np.nonzero(mask[b])[0] for b in range(BATCH)]
    ke_max = max(1, max(len(i) for i in idxs))
    k_pad = 384 * ((ke_max + 383) // 384)
    if k_pad > SEQ:
        k_pad = SEQ
    KC = 512 if k_pad % 512 == 0 else 384
    NKC = k_pad // KC
    KT_N = k_pad // 128

    ones16 = np.ones(128, F16)

    in_maps = []
    for b in range(BATCH):
        xT = x[b].T                                  # [D, S] view
        # xq: [qc, p, dt, 512]
        xq = np.ascontiguousarray(
            xT.reshape(8, 128, 4, 512).transpose(2, 1, 0, 3)).astype(F16)
        idx = idxs[b]
        ke = len(idx)
        if ke > k_pad:
            idx = idx[:k_pad]
            ke = k_pad
        xkT = np.zeros((D_MODEL, k_pad), np.float32)
        xkT[:, :ke] = x[b][idx].T
        # xk: [kc, p, dt, KC]
        xk = np.ascontiguousarray(
            xkT.reshape(8, 128, NKC, KC).transpose(2, 1, 0, 3)).astype(F16)
        maskb = np.zeros(k_pad, np.float32)
        maskb[ke:] = NEG
        mb_t = _tile_pf(maskb)                       # [128, KT_N]
        for g in range(2):
            fs = slice(g * FH, (g + 1) * FH)
            # Wv/bv padded with a ones column per head: the V-projection
            # matmul then produces [V_h | ones] directly (col = 0*x + 1.0).
            Wv_aug = np.zeros((D_MODEL, HPC * 65), np.float32)
            bv_aug = np.zeros(HPC * 65, np.float32)
            for h in range(HPC):
                Wv_aug[:, h * 65:h * 65 + 64] = Wv[:, g * FH + h * 64:
                                                   g * FH + (h + 1) * 64]
                bv_aug[h * 65:h * 65 + 64] = bv[g * FH + h * 64:
                                                g * FH + (h + 1) * 64]
                bv_aug[h * 65 + 64] = 1.0
            in_maps.append({
                "xq": xq,
                "xk": xk,
                "Wq": _tile_pf(np.asarray(Wq[:, fs], np.float32)).astype(F16),
                "Wk": _tile_pf(np.asarray(Wk[:, fs], np.float32)).astype(F16),
                "Wv": _tile_pf(Wv_aug).astype(F16),
                "Wo": _tile_pf(np.asarray(Wo[fs, :], np.float32)).astype(F16),
                "bcst": np.concatenate(
                    [_tile_pf(np.asarray(bq[fs], np.float32)),
                     _tile_pf(np.asarray(bk[fs], np.float32)),
                     mb_t], axis=1).astype(np.float32),
                "bv": bv_aug.astype(F16),
                "ones16": ones16,
            })
    return in_maps, k_pad


def kernel(x, attention_mask, Wq, bq, Wk, bk, Wv, bv, Wo, bo):
    global last_results
    from concourse.bass_utils import run_bass_kernel_spmd

    in_maps, k_pad = _prep_core_inputs(x, attention_mask, Wq, bq, Wk, bk,
                                       Wv, bv, Wo)
    nc = _get_compiled(k_pad)
    res = run_bass_kernel_spmd(nc, in_maps, core_ids=list(range(N_CORES)))
    last_results = res

    bo = np.asarray(bo, np.float32)
    out = np.empty((BATCH, SEQ, D_MODEL), np.float32)
    for b in range(BATCH):
        out[b] = res.results[2 * b]["out"] + res.results[2 * b + 1]["out"] + bo
    return out


# revision 18
# speedup vs baseline: 1.7180x; 1.0172x over previous
"""Multi-head attention kernel for 8 Trainium2 NeuronCores.

Problem: B=4, S=2048, D=1024, H=16, Dh=64 MHA with key-side boolean mask.

Sharding: core c handles (batch b = c//2, head-half g = c%2, 8 heads each).
QKV are column-parallel, the output projection is row-parallel (Megatron
style); the host sums the two partial output projections per batch and adds
the output bias.

Host-side preprocessing (pure data marshalling):
  - All inputs are pre-tiled into DMA-native layouts (partition-major,
    contiguous per partition).
  - x is transposed per batch (the PE contracts over the partition dim).
  - Keys with mask=False contribute exactly zero after softmax, so the host
    gathers only the unmasked keys (padded to a multiple of 384 with zero
    rows whose exp-bias is -1e30 => exp == 0 exactly).
  - All matmul operands are fp16 (same PE throughput as bf16 on TRN2 but
    8x lower quantization noise; attention averages ~1e3 near-uniform keys
    so per-element noise in E/V passes straight to the output).

On-core dataflow (all matmuls fp16, PSUM accumulation fp32):
  xT --(Wk)--> KT[f,k]             bias fused in the ScalarE PSUM->SBUF copy
  xT --(Wv)--> Vau[k, h, 65]       (aug ones col -> softmax denominator)
  xT --(Wq)--> QT[f,q]
  scores[k,q] = KT_h^T x QT_h      64-deep contraction at base partition
                                   0/64 (cost is column-count bound)
  E = exp(scores*0.125 + maskbias[k])   one ScalarE pass per key tile,
                                   written to SBUF as fp16; ScalarE does
                                   nothing else during attention
  av[65,q] += Vau_kt^T x E_kt      accumulated over key tiles in PSUM
  attnT[f,q] = av[0:64] * bcast(1/av[64])  (ones-matmul broadcast + DVE)
  out[s,D] = attnT^T x Wo          (partial; host adds pair + bo)
"""

import os
import numpy as np

os.environ.setdefault("MYCRO_LOCAL_CACHE", "1")

D_MODEL = 1024
N_HEADS = 16
D_HEAD = 64
BATCH = 4
SEQ = 2048
N_CORES = 8
FH = 512          # features per core (8 heads x 64)
HPC = 8           # heads per core
NEG = -1.0e30     # additive bias for padded/masked keys; exp -> 0 exactly

F16 = np.float16

_COMPILED = {}    # k_pad -> nc
last_results = None  # BassKernelResults of the most recent run (for test.py)


def _build(k_pad):
    """Emit + compile the per-core bass kernel for a given padded key count."""
    import concourse.bacc as bacc
    import concourse.bass as bass
    import concourse.tile as tile
    from concourse import mybir

    f32 = mybir.dt.float32
    f32r = mybir.dt.float32r
    f16 = mybir.dt.float16

    KT_N = k_pad // 128                     # number of 128-key tiles
    KC = 512 if k_pad % 512 == 0 else 384   # key-side chunk
    assert k_pad % KC == 0 and KC % 128 == 0
    NKC = k_pad // KC
    HW = HPC * 65   # augmented V width (520)

    nc = bacc.Bacc("TRN2", target_bir_lowering=False, debug=False,
                   num_devices=N_CORES)

    # all pre-tiled on host into DMA-native layouts
    dxq = nc.dram_tensor("xq", [4, 128, 8, 512], f16, kind="ExternalInput")
    dxk = nc.dram_tensor("xk", [NKC, 128, 8, KC], f16, kind="ExternalInput")
    dWq = nc.dram_tensor("Wq", [128, 8, FH], f16, kind="ExternalInput")
    dWk = nc.dram_tensor("Wk", [128, 8, FH], f16, kind="ExternalInput")
    dWv = nc.dram_tensor("Wv", [128, 8, HW], f16, kind="ExternalInput")
    dWo = nc.dram_tensor("Wo", [128, 4, D_MODEL], f16, kind="ExternalInput")
    dbc = nc.dram_tensor("bcst", [128, 8 + KT_N], f32, kind="ExternalInput")
    dbv = nc.dram_tensor("bv", [HW], f16, kind="ExternalInput")
    dc16 = nc.dram_tensor("ones16", [128], f16, kind="ExternalInput")
    drs = nc.dram_tensor("rscratch", [16, 1024], f16, kind="Internal")
    dout = nc.dram_tensor("out", [SEQ, D_MODEL], f32, kind="ExternalOutput")

    EXP = mybir.ActivationFunctionType.Exp
    IDn = mybir.ActivationFunctionType.Identity

    with tile.TileContext(nc) as tc:
        with tc.tile_pool(name="persist", bufs=1) as pers:
            # ---- constants in SBUF ----
            bc = pers.tile([128, 8 + KT_N], f32, tag="bcst")
            nc.sync.dma_start(out=bc, in_=dbc.ap())
            bq = bc[:, 0:4]
            bk = bc[:, 4:8]
            mb = bc[:, 8:8 + KT_N]
            bv_row = pers.tile([1, HW], f16, tag="bvr")
            nc.sync.dma_start(out=bv_row, in_=dbv.ap()[None, :])
            ones16 = pers.tile([1, 128], f16, tag="ones16")
            nc.sync.dma_start(out=ones16, in_=dc16.ap()[None, :])

            # ---- persistent activations ----
            QT = pers.tile([128, 4, SEQ], f16, tag="QT")         # [f, q]
            KT = pers.tile([128, 4, k_pad], f16, tag="KT")       # [f, k]
            Vau = pers.tile([128, KT_N, HPC, 65], f16, tag="Vau")
            attnT = pers.tile([128, 4, SEQ], f16, tag="attnT")   # [f, q]
            wo = pers.tile([128, 4, D_MODEL], f16, tag="wo")

            # ================= projections =================
            # (wq/xq DMAs are emitted after the K-side DMAs so the first
            # K-projection matmul isn't stuck behind 3MB of Q-side input)
            wq = pers.tile([128, 8, FH], f16, tag="wq")
            xq2a = pers.tile([128, 8, 512], f16, tag="xq2a")
            xq2b = pers.tile([128, 8, 512], f16, tag="xq2b")
            xq2 = {2: xq2a, 3: xq2b}
            ppool_cm = tc.tile_pool(name="pp", bufs=4, space="PSUM")
            ppool = ppool_cm.__enter__()

            # ----- K side (KT, V) -----
            with tc.tile_pool(name="wtk", bufs=1) as wtk, \
                 tc.tile_pool(name="xk", bufs=2) as xkp:
                pk = ppool
                wk = wtk.tile([128, 8, FH], f16, tag="wk")
                for d in range(8):
                    nc.sync.dma_start(out=wk[:, d, :], in_=dWk.ap()[:, d, :])
                wv = wtk.tile([128, 8, HW], f16, tag="wv")
                nc.sync.dma_start(out=wv, in_=dWv.ap())
                for kc in range(NKC):
                    xk_t = xkp.tile([128, 8, KC], f16, tag="xk")
                    for d in range(8):
                        nc.sync.dma_start(out=xk_t[:, d, :],
                                          in_=dxk.ap()[kc][:, d, :])
                    for ft in range(4):
                        ps = pk.tile([128, KC], f32, tag="pk")
                        for d in range(8):
                            nc.tensor.matmul(
                                ps,
                                lhsT=wk[:, d, ft * 128:(ft + 1) * 128],
                                rhs=xk_t[:, d, :],
                                start=(d == 0), stop=(d == 7))
                        ks = slice(kc * KC, (kc + 1) * KC)
                        nc.scalar.activation(KT[:, ft, ks], ps, IDn,
                                             bias=bk[:, ft:ft + 1])
                    for kb in range(KC // 128):
                        kg = kc * (KC // 128) + kb
                        ps = pk.tile([128, HW], f32, tag="pk")
                        for d in range(8):
                            lt = xk_t[:, d, kb * 128:(kb + 1) * 128]
                            nc.tensor.matmul(
                                ps[:, 0:512], lhsT=lt,
                                rhs=wv[:, d, 0:512],
                                start=(d == 0), stop=False)
                            nc.tensor.matmul(
                                ps[:, 512:520], lhsT=lt,
                                rhs=wv[:, d, 512:520],
                                start=(d == 0), stop=False)
                        nc.tensor.matmul(ps[:, 0:512], lhsT=ones16,
                                         rhs=bv_row[:, 0:512],
                                         start=False, stop=True)
                        nc.tensor.matmul(ps[:, 512:520], lhsT=ones16,
                                         rhs=bv_row[:, 512:520],
                                         start=False, stop=True)
                        nc.scalar.copy(Vau[:, kg, :, :], ps)

            # ----- Q side (QT): qc 0,1 here; qc 2,3 woven into the
            # qh=0 attention loop (their queries are only read in qh=1)
            nc.sync.dma_start(out=wq, in_=dWq.ap())
            nc.sync.dma_start(out=xq2a, in_=dxq.ap()[2])
            nc.sync.dma_start(out=xq2b, in_=dxq.ap()[3])
            with tc.tile_pool(name="xq", bufs=2) as xqp:
                pq = ppool
                for qc in range(2):
                    xq_t = xqp.tile([128, 8, 512], f16, tag="xq")
                    nc.sync.dma_start(out=xq_t, in_=dxq.ap()[qc])
                    for ft in range(4):
                        ps = pq.tile([128, 512], f32, tag="pk")
                        for d in range(8):
                            nc.tensor.matmul(
                                ps,
                                lhsT=wq[:, d, ft * 128:(ft + 1) * 128],
                                rhs=xq_t[:, d, :],
                                start=(d == 0), stop=(d == 7))
                        nc.scalar.activation(QT[:, ft, qc * 512:(qc + 1) * 512],
                                             ps, IDn, bias=bq[:, ft:ft + 1])

            ppool_cm.__exit__(None, None, None)
            nc.sync.dma_start(out=wo, in_=dWo.ap())

            # ================= attention core =================
            # Per (qh, t, h): KT_N score tiles [128k, 1024q] through a
            # double-buffered PSUM pool; exp each tile straight to fp16 E
            # in SBUF; AV accumulates over key tiles in PSUM. ScalarE does
            # only exp here. O-projection matmuls for the finished query
            # half are woven one-per-key-tile-slot into the other half's
            # attention loop, keeping the PE busy (full p-state) while it
            # would otherwise wait on ScalarE.
            with tc.tile_pool(name="ep", bufs=2) as epl, \
                 tc.tile_pool(name="up", bufs=2) as upl, \
                 tc.tile_pool(name="rp", bufs=2) as rpl, \
                 tc.tile_pool(name="sp", bufs=2, space="PSUM") as spl, \
                 tc.tile_pool(name="av", bufs=1, space="PSUM") as avl, \
                 tc.tile_pool(name="op", bufs=2, space="PSUM") as opl, \
                 tc.tile_pool(name="ot", bufs=3) as otl:

                def oproj_tile(st, tail=False):
                    """Yield (emit-)closures: 8 matmul slots + finalizers.
                    In the tail (no exp running) the PSUM->SBUF copies
                    alternate between ScalarE and DVE."""
                    sts = slice(st * 128, (st + 1) * 128)
                    ps = [None, None]

                    def mk_mm(dh, ft):
                        def mm():
                            if ft == 0:
                                ps[dh] = opl.tile([128, 512], f32, tag="op",
                                                  name=f"ops{st}_{dh}")
                            nc.tensor.matmul(
                                ps[dh],
                                lhsT=attnT[:, ft, sts],
                                rhs=wo[:, ft, dh * 512:(dh + 1) * 512],
                                start=(ft == 0), stop=(ft == 3))
                            if ft == 3:
                                ot = otl.tile([128, 512], f32, tag="ot")
                                if tail and dh == 0:
                                    nc.scalar.copy(ot, ps[dh])
                                else:
                                    nc.vector.tensor_copy(ot, ps[dh])
                                nc.sync.dma_start(
                                    out=dout.ap()[sts,
                                                  dh * 512:(dh + 1) * 512],
                                    in_=ot)
                        return mm

                    return [mk_mm(dh, ft) for dh in range(2)
                            for ft in range(4)]

                def qproj_tile(qc, ft):
                    """8 matmul closures accumulating one QT ft-chunk."""
                    ps = [None]

                    def mk_mm(d):
                        def mm():
                            if d == 0:
                                ps[0] = opl.tile([128, 512], f32, tag="op",
                                                 name=f"qps{qc}_{ft}")
                            nc.tensor.matmul(
                                ps[0],
                                lhsT=wq[:, d, ft * 128:(ft + 1) * 128],
                                rhs=xq2[qc][:, d, :],
                                start=(d == 0), stop=(d == 7))
                            if d == 7:
                                nc.scalar.activation(
                                    QT[:, ft, qc * 512:(qc + 1) * 512],
                                    ps[0], IDn, bias=bq[:, ft:ft + 1])
                        return mm

                    return [mk_mm(d) for d in range(8)]

                def attn_head(qh, t, h, weave):
                    q0 = qh * 1024
                    p0 = h * 64
                    hh = 2 * t + h
                    E = epl.tile([128, KT_N, 1024], f16, tag="E")
                    av = avl.tile([65, 1024], f32, tag="av")

                    def scores_exp(kt):
                        s = spl.tile([128, 1024], f32, tag="s")
                        kts = slice(kt * 128, (kt + 1) * 128)
                        for c in range(2):
                            cs = slice(c * 512, (c + 1) * 512)
                            qs = slice(q0 + c * 512, q0 + (c + 1) * 512)
                            nc.tensor.matmul(
                                s[:, cs],
                                lhsT=KT[p0:p0 + 64, t, kts],
                                rhs=QT[p0:p0 + 64, t, qs],
                                start=True, stop=True)
                        nc.scalar.activation(
                            E[:, kt, :], s, EXP,
                            bias=mb[:, kt:kt + 1], scale=0.125)

                    # software pipeline: scores/exp run one tile ahead of
                    # the AV consumer; weave ops fill the exp-wait gap
                    scores_exp(0)
                    wv_i = 0
                    wv_n = len(weave)
                    for kt in range(KT_N):
                        if kt + 1 < KT_N:
                            scores_exp(kt + 1)
                        while wv_i < wv_n and wv_i * KT_N < (kt + 1) * wv_n:
                            weave[wv_i]()
                            wv_i += 1
                        for c in range(2):
                            cs = slice(c * 512, (c + 1) * 512)
                            nc.tensor.matmul(
                                av[:, cs],
                                lhsT=Vau[:, kt, hh, :],
                                rhs=E[:, kt, cs],
                                start=(kt == 0),
                                stop=(kt == KT_N - 1))

                    # normalize: attnT = av[0:64] * bcast(1/av[64]).
                    # u-copy is emitted first so the av PSUM tile frees
                    # for the next head before the recip chain drains.
                    dn = rpl.tile([1, 1024], f32, tag="dn")
                    nc.scalar.copy(dn, av[64:65, :])
                    u = upl.tile([64, 1024], f16, tag="u")
                    with nc.allow_low_precision(reason="fp16 attn staging"):
                        nc.vector.tensor_copy(u, av[0:64, :])
                    rf = rpl.tile([1, 1024], f32, tag="rf")
                    nc.vector.reciprocal_approx_fast(out=rf, in_=dn)
                    r16 = rpl.tile([1, 1024], f16, tag="r16")
                    with nc.allow_low_precision(reason="fp16 recip"):
                        nc.vector.tensor_copy(r16, rf)
                    # broadcast 1/den across 64 partitions via a DRAM
                    # bounce with a stride-0-partition read (keeps the
                    # score PSUM pool free of normalize traffic, so the
                    # exp pipeline flows across head boundaries)
                    ri = (qh * 8 + t * 2 + h)
                    nc.sync.dma_start(out=drs.ap()[ri][None, :], in_=r16)
                    bc16 = upl.tile([64, 1024], f16, tag="bc")
                    nc.sync.dma_start(
                        out=bc16,
                        in_=bass.AP(tensor=drs.ap().tensor,
                                    offset=ri * 1024,
                                    ap=[[0, 64], [1, 1024]]))
                    with nc.allow_low_precision(
                            reason="fp16 attn staging"):
                        nc.vector.tensor_mul(
                            attnT[p0:p0 + 64, t, q0:q0 + 1024],
                            u, bc16)

                qweave = [qproj_tile(qc, ft)
                          for qc in (2, 3) for ft in range(4)]
                for i, (t, h) in enumerate(
                        [(t, h) for t in range(4) for h in range(2)]):
                    attn_head(0, t, h, qweave[i])
                pending = []
                for i, (t, h) in enumerate(
                        [(t, h) for t in range(4) for h in range(2)]):
                    pending += oproj_tile(i)      # q-half 0 output tiles
                    attn_head(1, t, h, pending)
                    pending = []
                # tail: q-half 1 output tiles
                for st in range(8, 16):
                    for mm in oproj_tile(st, tail=True):
                        mm()

    nc.compile()
    return nc


def _get_compiled(k_pad):
    if k_pad not in _COMPILED:
        _COMPILED[k_pad] = _build(k_pad)
    return _COMPILED[k_pad]


def _tile_pf(a, p=128):
    """[P*t, f...] -> contiguous [p, t, f...] partition-major tiling."""
    t = a.shape[0] // p
    return np.ascontiguousarray(
        a.reshape(t, p, *a.shape[1:]).swapaxes(0, 1))


def _prep_core_inputs(x, attention_mask, Wq, bq, Wk, bk, Wv, bv, Wo):
    """Host-side shard prep. Returns (in_maps, k_pad)."""
    x = np.asarray(x, np.float32)
    mask = np.asarray(attention_mask, bool)
    idxs = [np.nonzero(mask[b])[0] for b in range(BATCH)]
    ke_max = max(1, max(len(i) for i in idxs))
    k_pad = 384 * ((ke_max + 383) // 384)
    if k_pad > SEQ:
        k_pad = SEQ
    KC = 512 if k_pad % 512 == 0 else 384
    NKC = k_pad // KC
    KT_N = k_pad // 128

    ones16 = np.ones(128, F16)

    in_maps = []
    for b in range(BATCH):
        xT = x[b].T                                  # [D, S] view
        # xq: [qc, p, dt, 512]
        xq = np.ascontiguousarray(
            xT.reshape(8, 128, 4, 512).transpose(2, 1, 0, 3)).astype(F16)
        idx = idxs[b]
        ke = len(idx)
        if ke > k_pad:
            idx = idx[:k_pad]
            ke = k_pad
        xkT = np.zeros((D_MODEL, k_pad), np.float32)
        xkT[:, :ke] = x[b][idx].T
        # xk: [kc, p, dt, KC]
        xk = np.ascontiguousarray(
            xkT.reshape(8, 128, NKC, KC).transpose(2, 1, 0, 3)).astype(F16)
        maskb = np.zeros(k_pad, np.float32)
        maskb[ke:] = NEG
        mb_t = _tile_pf(maskb)                       # [128, KT_N]
        for g in range(2):
            fs = slice(g * FH, (g + 1) * FH)
            # Wv/bv padded with a ones column per head: the V-projection
            # matmul then produces [V_h | ones] directly (col = 0*x + 1.0).
            Wv_aug = np.zeros((D_MODEL, HPC * 65), np.float32)
            bv_aug = np.zeros(HPC * 65, np.float32)
            for h in range(HPC):
                Wv_aug[:, h * 65:h * 65 + 64] = Wv[:, g * FH + h * 64:
                                                   g * FH + (h + 1) * 64]
                bv_aug[h * 65:h * 65 + 64] = bv[g * FH + h * 64:
                                                g * FH + (h + 1) * 64]
                bv_aug[h * 65 + 64] = 1.0
            in_maps.append({
                "xq": xq,
                "xk": xk,
                "Wq": _tile_pf(np.asarray(Wq[:, fs], np.float32)).astype(F16),
                "Wk": _tile_pf(np.asarray(Wk[:, fs], np.float32)).astype(F16),
                "Wv": _tile_pf(Wv_aug).astype(F16),
                "Wo": _tile_pf(np.asarray(Wo[fs, :], np.float32)).astype(F16),
                "bcst": np.concatenate(
                    [_tile_pf(np.asarray(bq[fs], np.float32)),
                     _tile_pf(np.asarray(bk[fs], np.float32)),
                     mb_t], axis=1).astype(np.float32),
                "bv": bv_aug.astype(F16),
                "ones16": ones16,
            })
    return in_maps, k_pad


def kernel(x, attention_mask, Wq, bq, Wk, bk, Wv, bv, Wo, bo):
    global last_results
    from concourse.bass_utils import run_bass_kernel_spmd

    in_maps, k_pad = _prep_core_inputs(x, attention_mask, Wq, bq, Wk, bk,
                                       Wv, bv, Wo)
    nc = _get_compiled(k_pad)
    res = run_bass_kernel_spmd(nc, in_maps, core_ids=list(range(N_CORES)))
    last_results = res

    bo = np.asarray(bo, np.float32)
    out = np.empty((BATCH, SEQ, D_MODEL), np.float32)
    for b in range(BATCH):
        out[b] = res.results[2 * b]["out"] + res.results[2 * b + 1]["out"] + bo
    return out


# revision 19
# speedup vs baseline: 1.8329x; 1.0668x over previous
"""Multi-head attention kernel for 8 Trainium2 NeuronCores.

Problem: B=4, S=2048, D=1024, H=16, Dh=64 MHA with key-side boolean mask.

Sharding: core c handles (batch b = c//2, head-half g = c%2, 8 heads each).
QKV are column-parallel, the output projection is row-parallel (Megatron
style); the host sums the two partial output projections per batch and adds
the output bias.

Host-side preprocessing (pure data marshalling):
  - All inputs are pre-tiled into DMA-native layouts (partition-major,
    contiguous per partition).
  - x is transposed per batch (the PE contracts over the partition dim).
  - Keys with mask=False contribute exactly zero after softmax, so the host
    gathers only the unmasked keys (padded to a multiple of 384 with zero
    rows whose exp-bias is -1e30 => exp == 0 exactly).
  - All matmul operands are fp16 (same PE throughput as bf16 on TRN2 but
    8x lower quantization noise; attention averages ~1e3 near-uniform keys
    so per-element noise in E/V passes straight to the output).

On-core dataflow (all matmuls fp16, PSUM accumulation fp32):
  xT --(Wk)--> KT[f,k]             bias fused in the ScalarE PSUM->SBUF copy
  xT --(Wv)--> Vau[k, h, 65]       (aug ones col -> softmax denominator)
  xT --(Wq)--> QT[f,q]
  scores[k,q] = KT_h^T x QT_h      64-deep contraction at base partition
                                   0/64 (cost is column-count bound)
  E = exp(scores*0.125 + maskbias[k])   one ScalarE pass per key tile,
                                   written to SBUF as fp16; ScalarE does
                                   nothing else during attention
  av[65,q] += Vau_kt^T x E_kt      accumulated over key tiles in PSUM
  attnT[f,q] = av[0:64] * bcast(1/av[64])  (ones-matmul broadcast + DVE)
  out[s,D] = attnT^T x Wo          (partial; host adds pair + bo)
"""

import os
import numpy as np

os.environ.setdefault("MYCRO_LOCAL_CACHE", "1")

D_MODEL = 1024
N_HEADS = 16
D_HEAD = 64
BATCH = 4
SEQ = 2048
N_CORES = 8
FH = 512          # features per core (8 heads x 64)
HPC = 8           # heads per core
NEG = -1.0e30     # additive bias for padded/masked keys; exp -> 0 exactly

F16 = np.float16

_COMPILED = {}    # k_pad -> nc
last_results = None  # BassKernelResults of the most recent run (for test.py)


def _build(k_pad):
    """Emit + compile the per-core bass kernel for a given padded key count."""
    import concourse.bacc as bacc
    import concourse.bass as bass
    import concourse.tile as tile
    from concourse import mybir

    f32 = mybir.dt.float32
    f32r = mybir.dt.float32r
    f16 = mybir.dt.float16

    KT_N = k_pad // 128                     # number of 128-key tiles
    KC = 512 if k_pad % 512 == 0 else 384   # key-side chunk
    assert k_pad % KC == 0 and KC % 128 == 0
    NKC = k_pad // KC
    HW = HPC * 65   # augmented V width (520)

    nc = bacc.Bacc("TRN2", target_bir_lowering=False, debug=False,
                   num_devices=N_CORES)

    # all pre-tiled on host into DMA-native layouts
    dxq = nc.dram_tensor("xq", [4, 128, 8, 512], f16, kind="ExternalInput")
    dxk = nc.dram_tensor("xk", [NKC, 128, 8, KC], f16, kind="ExternalInput")
    dWq = nc.dram_tensor("Wq", [128, 8, FH], f16, kind="ExternalInput")
    dWk = nc.dram_tensor("Wk", [128, 8, FH], f16, kind="ExternalInput")
    dWv = nc.dram_tensor("Wv", [128, 8, HW], f16, kind="ExternalInput")
    dWo = nc.dram_tensor("Wo", [128, 4, D_MODEL], f16, kind="ExternalInput")
    dbc = nc.dram_tensor("bcst", [128, 8 + KT_N], f32, kind="ExternalInput")
    dbv = nc.dram_tensor("bv", [HW], f16, kind="ExternalInput")
    dc16 = nc.dram_tensor("ones16", [128], f16, kind="ExternalInput")
    drs = nc.dram_tensor("rscratch", [16, 1024], f16, kind="Internal")
    dout = nc.dram_tensor("out", [SEQ, D_MODEL], f32, kind="ExternalOutput")

    EXP = mybir.ActivationFunctionType.Exp
    IDn = mybir.ActivationFunctionType.Identity

    with tile.TileContext(nc) as tc:
        with tc.tile_pool(name="persist", bufs=1) as pers:
            # ---- constants in SBUF ----
            bc = pers.tile([128, 8 + KT_N], f32, tag="bcst")
            nc.sync.dma_start(out=bc, in_=dbc.ap())
            bq = bc[:, 0:4]
            bk = bc[:, 4:8]
            mb = bc[:, 8:8 + KT_N]
            bv_row = pers.tile([1, HW], f16, tag="bvr")
            nc.sync.dma_start(out=bv_row, in_=dbv.ap()[None, :])
            ones16 = pers.tile([1, 128], f16, tag="ones16")
            nc.sync.dma_start(out=ones16, in_=dc16.ap()[None, :])

            # ---- persistent activations ----
            QT = pers.tile([128, 4, SEQ], f16, tag="QT")         # [f, q]
            KT = pers.tile([128, 4, k_pad], f16, tag="KT")       # [f, k]
            Vau = pers.tile([128, KT_N, HPC, 65], f16, tag="Vau")
            attnT = pers.tile([128, 4, SEQ], f16, tag="attnT")   # [f, q]
            wo = pers.tile([128, 4, D_MODEL], f16, tag="wo")

            # ================= projections =================
            # (wq/xq DMAs are emitted after the K-side DMAs so the first
            # K-projection matmul isn't stuck behind 3MB of Q-side input)
            wq = pers.tile([128, 8, FH], f16, tag="wq")
            xq2a = pers.tile([128, 8, 512], f16, tag="xq2a")
            xq2b = pers.tile([128, 8, 512], f16, tag="xq2b")
            xq2 = {2: xq2a, 3: xq2b}
            ppool_cm = tc.tile_pool(name="pp", bufs=4, space="PSUM")
            ppool = ppool_cm.__enter__()

            # ----- K side (KT, V) -----
            with tc.tile_pool(name="wtk", bufs=1) as wtk, \
                 tc.tile_pool(name="xk", bufs=2) as xkp:
                pk = ppool
                wk = wtk.tile([128, 8, FH], f16, tag="wk")
                wv = wtk.tile([128, 8, HW], f16, tag="wv")
                xk_first = [None]
                for kc in range(NKC):
                    if kc == 0:
                        # interleave wk/xk d-chunks so the d=0 pair (the
                        # first matmul's operands) lands first; the 1MB wv
                        # transfer (needed ~7us later) follows them
                        xk_t = xkp.tile([128, 8, KC], f16, tag="xk")
                        for d in range(8):
                            nc.sync.dma_start(out=wk[:, d, :],
                                              in_=dWk.ap()[:, d, :])
                            nc.sync.dma_start(out=xk_t[:, d, :],
                                              in_=dxk.ap()[0][:, d, :])
                        nc.sync.dma_start(out=wv, in_=dWv.ap())
                    else:
                        xk_t = xkp.tile([128, 8, KC], f16, tag="xk")
                        for d in range(8):
                            nc.sync.dma_start(out=xk_t[:, d, :],
                                              in_=dxk.ap()[kc][:, d, :])
                    for ft in range(4):
                        ps = pk.tile([128, KC], f32, tag="pk")
                        for d in range(8):
                            nc.tensor.matmul(
                                ps,
                                lhsT=wk[:, d, ft * 128:(ft + 1) * 128],
                                rhs=xk_t[:, d, :],
                                start=(d == 0), stop=(d == 7))
                        ks = slice(kc * KC, (kc + 1) * KC)
                        nc.scalar.activation(KT[:, ft, ks], ps, IDn,
                                             bias=bk[:, ft:ft + 1])
                    for kb in range(KC // 128):
                        kg = kc * (KC // 128) + kb
                        ps = pk.tile([128, HW], f32, tag="pk")
                        for d in range(8):
                            lt = xk_t[:, d, kb * 128:(kb + 1) * 128]
                            nc.tensor.matmul(
                                ps[:, 0:512], lhsT=lt,
                                rhs=wv[:, d, 0:512],
                                start=(d == 0), stop=False)
                            nc.tensor.matmul(
                                ps[:, 512:520], lhsT=lt,
                                rhs=wv[:, d, 512:520],
                                start=(d == 0), stop=False)
                        nc.tensor.matmul(ps[:, 0:512], lhsT=ones16,
                                         rhs=bv_row[:, 0:512],
                                         start=False, stop=True)
                        nc.tensor.matmul(ps[:, 512:520], lhsT=ones16,
                                         rhs=bv_row[:, 512:520],
                                         start=False, stop=True)
                        nc.scalar.copy(Vau[:, kg, :, :], ps)

            # ----- Q side (QT): qc 0,1 here; qc 2,3 woven into the
            # qh=0 attention loop (their queries are only read in qh=1)
            nc.sync.dma_start(out=wq, in_=dWq.ap())
            nc.sync.dma_start(out=xq2a, in_=dxq.ap()[2])
            nc.sync.dma_start(out=xq2b, in_=dxq.ap()[3])
            with tc.tile_pool(name="xq", bufs=2) as xqp:
                pq = ppool
                for qc in range(2):
                    xq_t = xqp.tile([128, 8, 512], f16, tag="xq")
                    nc.sync.dma_start(out=xq_t, in_=dxq.ap()[qc])
                    for ft in range(4):
                        ps = pq.tile([128, 512], f32, tag="pk")
                        for d in range(8):
                            nc.tensor.matmul(
                                ps,
                                lhsT=wq[:, d, ft * 128:(ft + 1) * 128],
                                rhs=xq_t[:, d, :],
                                start=(d == 0), stop=(d == 7))
                        nc.scalar.activation(QT[:, ft, qc * 512:(qc + 1) * 512],
                                             ps, IDn, bias=bq[:, ft:ft + 1])

            ppool_cm.__exit__(None, None, None)
            nc.sync.dma_start(out=wo, in_=dWo.ap())

            # ================= attention core =================
            # Per (qh, t, h): KT_N score tiles [128k, 1024q] through a
            # double-buffered PSUM pool; exp each tile straight to fp16 E
            # in SBUF; AV accumulates over key tiles in PSUM. ScalarE does
            # only exp here. O-projection matmuls for the finished query
            # half are woven one-per-key-tile-slot into the other half's
            # attention loop, keeping the PE busy (full p-state) while it
            # would otherwise wait on ScalarE.
            with tc.tile_pool(name="ep", bufs=2) as epl, \
                 tc.tile_pool(name="up", bufs=2) as upl, \
                 tc.tile_pool(name="rp", bufs=2) as rpl, \
                 tc.tile_pool(name="sp", bufs=2, space="PSUM") as spl, \
                 tc.tile_pool(name="av", bufs=1, space="PSUM") as avl, \
                 tc.tile_pool(name="op", bufs=2, space="PSUM") as opl, \
                 tc.tile_pool(name="ot", bufs=3) as otl:

                def oproj_tile(st, tail=False):
                    """Yield (emit-)closures: 8 matmul slots + finalizers.
                    In the tail (no exp running) the PSUM->SBUF copies
                    alternate between ScalarE and DVE."""
                    sts = slice(st * 128, (st + 1) * 128)
                    ps = [None, None]

                    def mk_mm(dh, ft):
                        def mm():
                            if ft == 0:
                                ps[dh] = opl.tile([128, 512], f32, tag="op",
                                                  name=f"ops{st}_{dh}")
                            nc.tensor.matmul(
                                ps[dh],
                                lhsT=attnT[:, ft, sts],
                                rhs=wo[:, ft, dh * 512:(dh + 1) * 512],
                                start=(ft == 0), stop=(ft == 3))
                            if ft == 3:
                                ot = otl.tile([128, 512], f32, tag="ot")
                                if tail and dh == 0:
                                    nc.scalar.copy(ot, ps[dh])
                                else:
                                    nc.vector.tensor_copy(ot, ps[dh])
                                nc.sync.dma_start(
                                    out=dout.ap()[sts,
                                                  dh * 512:(dh + 1) * 512],
                                    in_=ot)
                        return mm

                    return [mk_mm(dh, ft) for dh in range(2)
                            for ft in range(4)]

                def qproj_tile(qc, ft):
                    """8 matmul closures accumulating one QT ft-chunk."""
                    ps = [None]

                    def mk_mm(d):
                        def mm():
                            if d == 0:
                                ps[0] = opl.tile([128, 512], f32, tag="op",
                                                 name=f"qps{qc}_{ft}")
                            nc.tensor.matmul(
                                ps[0],
                                lhsT=wq[:, d, ft * 128:(ft + 1) * 128],
                                rhs=xq2[qc][:, d, :],
                                start=(d == 0), stop=(d == 7))
                            if d == 7:
                                nc.scalar.activation(
                                    QT[:, ft, qc * 512:(qc + 1) * 512],
                                    ps[0], IDn, bias=bq[:, ft:ft + 1])
                        return mm

                    return [mk_mm(d) for d in range(8)]

                def attn_head(qh, t, h, weave):
                    q0 = qh * 1024
                    p0 = h * 64
                    hh = 2 * t + h
                    E = epl.tile([128, KT_N, 1024], f16, tag="E")
                    av = avl.tile([65, 1024], f32, tag="av")

                    def scores_exp(kt):
                        s = spl.tile([128, 1024], f32, tag="s")
                        kts = slice(kt * 128, (kt + 1) * 128)
                        for c in range(2):
                            cs = slice(c * 512, (c + 1) * 512)
                            qs = slice(q0 + c * 512, q0 + (c + 1) * 512)
                            nc.tensor.matmul(
                                s[:, cs],
                                lhsT=KT[p0:p0 + 64, t, kts],
                                rhs=QT[p0:p0 + 64, t, qs],
                                start=True, stop=True)
                        nc.scalar.activation(
                            E[:, kt, :], s, EXP,
                            bias=mb[:, kt:kt + 1], scale=0.125)

                    # software pipeline: scores/exp run one tile ahead of
                    # the AV consumer; weave ops fill the exp-wait gap
                    scores_exp(0)
                    wv_i = 0
                    wv_n = len(weave)
                    for kt in range(KT_N):
                        if kt + 1 < KT_N:
                            scores_exp(kt + 1)
                        while wv_i < wv_n and wv_i * KT_N < (kt + 1) * wv_n:
                            weave[wv_i]()
                            wv_i += 1
                        for c in range(2):
                            cs = slice(c * 512, (c + 1) * 512)
                            nc.tensor.matmul(
                                av[:, cs],
                                lhsT=Vau[:, kt, hh, :],
                                rhs=E[:, kt, cs],
                                start=(kt == 0),
                                stop=(kt == KT_N - 1))

                    # normalize: attnT = av[0:64] * bcast(1/av[64]).
                    # u-copy is emitted first so the av PSUM tile frees
                    # for the next head before the recip chain drains.
                    dn = rpl.tile([1, 1024], f32, tag="dn")
                    nc.vector.tensor_copy(dn, av[64:65, :])
                    u = upl.tile([64, 1024], f16, tag="u")
                    with nc.allow_low_precision(reason="fp16 attn staging"):
                        nc.vector.tensor_copy(u, av[0:64, :])
                    rf = rpl.tile([1, 1024], f32, tag="rf")
                    nc.vector.reciprocal_approx_fast(out=rf, in_=dn)
                    r16 = rpl.tile([1, 1024], f16, tag="r16")
                    with nc.allow_low_precision(reason="fp16 recip"):
                        nc.vector.tensor_copy(r16, rf)
                    # broadcast 1/den across 64 partitions via a DRAM
                    # bounce with a stride-0-partition read (keeps the
                    # score PSUM pool free of normalize traffic, so the
                    # exp pipeline flows across head boundaries)
                    ri = (qh * 8 + t * 2 + h)
                    nc.sync.dma_start(out=drs.ap()[ri][None, :], in_=r16)
                    bc16 = upl.tile([64, 1024], f16, tag="bc")
                    nc.sync.dma_start(
                        out=bc16,
                        in_=bass.AP(tensor=drs.ap().tensor,
                                    offset=ri * 1024,
                                    ap=[[0, 64], [1, 1024]]))
                    with nc.allow_low_precision(
                            reason="fp16 attn staging"):
                        nc.vector.tensor_mul(
                            attnT[p0:p0 + 64, t, q0:q0 + 1024],
                            u, bc16)

                qweave = [qproj_tile(qc, ft)
                          for qc in (2, 3) for ft in range(4)]
                for i, (t, h) in enumerate(
                        [(t, h) for t in range(4) for h in range(2)]):
                    attn_head(0, t, h, qweave[i])
                pending = []
                for i, (t, h) in enumerate(
                        [(t, h) for t in range(4) for h in range(2)]):
                    pending += oproj_tile(i)      # q-half 0 output tiles
                    attn_head(1, t, h, pending)
                    pending = []
                # tail: q-half 1 output tiles
                for st in range(8, 16):
                    for mm in oproj_tile(st, tail=True):
                        mm()

    nc.compile()
    return nc


def _get_compiled(k_pad):
    if k_pad not in _COMPILED:
        _COMPILED[k_pad] = _build(k_pad)
    return _COMPILED[k_pad]


def _tile_pf(a, p=128):
    """[P*t, f...] -> contiguous [p, t, f...] partition-major tiling."""
    t = a.shape[0] // p
    return np.ascontiguousarray(
        a.reshape(t, p, *a.shape[1:]).swapaxes(0, 1))


def _prep_core_inputs(x, attention_mask, Wq, bq, Wk, bk, Wv, bv, Wo):
    """Host-side shard prep. Returns (in_maps, k_pad)."""
    x = np.asarray(x, np.float32)
    mask = np.asarray(attention_mask, bool)
    idxs = [np.nonzero(mask[b])[0] for b in range(BATCH)]
    ke_max = max(1, max(len(i) for i in idxs))
    k_pad = 384 * ((ke_max + 383) // 384)
    if k_pad > SEQ:
        k_pad = SEQ
    KC = 512 if k_pad % 512 == 0 else 384
    NKC = k_pad // KC
    KT_N = k_pad // 128

    ones16 = np.ones(128, F16)

    in_maps = []
    for b in range(BATCH):
        xT = x[b].T                                  # [D, S] view
        # xq: [qc, p, dt, 512]
        xq = np.ascontiguousarray(
            xT.reshape(8, 128, 4, 512).transpose(2, 1, 0, 3)).astype(F16)
        idx = idxs[b]
        ke = len(idx)
        if ke > k_pad:
            idx = idx[:k_pad]
            ke = k_pad
        xkT = np.zeros((D_MODEL, k_pad), np.float32)
        xkT[:, :ke] = x[b][idx].T
        # xk: [kc, p, dt, KC]
        xk = np.ascontiguousarray(
            xkT.reshape(8, 128, NKC, KC).transpose(2, 1, 0, 3)).astype(F16)
        maskb = np.zeros(k_pad, np.float32)
        maskb[ke:] = NEG
        mb_t = _tile_pf(maskb)                       # [128, KT_N]
        for g in range(2):
            fs = slice(g * FH, (g + 1) * FH)
            # Wv/bv padded with a ones column per head: the V-projection
            # matmul then produces [V_h | ones] directly (col = 0*x + 1.0).
            Wv_aug = np.zeros((D_MODEL, HPC * 65), np.float32)
            bv_aug = np.zeros(HPC * 65, np.float32)
            for h in range(HPC):
                Wv_aug[:, h * 65:h * 65 + 64] = Wv[:, g * FH + h * 64:
                                                   g * FH + (h + 1) * 64]
                bv_aug[h * 65:h * 65 + 64] = bv[g * FH + h * 64:
                                                g * FH + (h + 1) * 64]
                bv_aug[h * 65 + 64] = 1.0
            in_maps.append({
                "xq": xq,
                "xk": xk,
                "Wq": _tile_pf(np.asarray(Wq[:, fs], np.float32)).astype(F16),
                "Wk": _tile_pf(np.asarray(Wk[:, fs], np.float32)).astype(F16),
                "Wv": _tile_pf(Wv_aug).astype(F16),
                "Wo": _tile_pf(np.asarray(Wo[fs, :], np.float32)).astype(F16),
                "bcst": np.concatenate(
                    [_tile_pf(np.asarray(bq[fs], np.float32)),
                     _tile_pf(np.asarray(bk[fs], np.float32)),
                     mb_t], axis=1).astype(np.float32),
                "bv": bv_aug.astype(F16),
                "ones16": ones16,
            })
    return in_maps, k_pad


def kernel(x, attention_mask, Wq, bq, Wk, bk, Wv, bv, Wo, bo):
    global last_results
    from concourse.bass_utils import run_bass_kernel_spmd

    in_maps, k_pad = _prep_core_inputs(x, attention_mask, Wq, bq, Wk, bk,
                                       Wv, bv, Wo)
    nc = _get_compiled(k_pad)
    res = run_bass_kernel_spmd(nc, in_maps, core_ids=list(range(N_CORES)))
    last_results = res

    bo = np.asarray(bo, np.float32)
    out = np.empty((BATCH, SEQ, D_MODEL), np.float32)
    for b in range(BATCH):
        out[b] = res.results[2 * b]["out"] + res.results[2 * b + 1]["out"] + bo
    return out
